# revision 1
# baseline (speedup 1.0000x reference)
"""Dense-MoE FFN kernel for TRN2, expert-parallel over 8 NeuronCores with
sparse token dispatch.

Reference computation (token t, hidden H=1024, ffn F=4096, E=8 experts,
top-K=2 routing):
    y_e = gelu_tanh(x @ w1_e + b1_e) @ w2_e + b2_e     (reference runs dense)
    weight[t, e] = sum_k probs[k, t] * (experts[k, t] == e)
    out[t] = sum_e weight[t, e] * y_e[t]

Tokens with weight[t, e] == 0 contribute exactly 0 to the sum, so each
expert only needs its routed tokens (~T*K/E plus dedupe, ~960 of 4096).

Sharding: expert-parallel. Core c holds expert c's weights. The host routes:
it gathers core c's tokens (x columns + combine weights) into a fixed
capacity-1024 buffer, the device computes
    out_sel[t'] = weight[t'] * (gelu_tanh(x_sel[t'] @ w1 + b1) @ w2 + b2)
and the host scatter-adds the partials into the full output (the unshard /
"all-reduce" step of the masked sum). If routing ever exceeds capacity,
larger variants of the same kernel (1280, then all-4096-tokens) keep the
result correct for any input distribution.

Device kernel (per core):
  - x arrives pre-gathered, pre-transposed (xT_sel [H, CAP]) and fp16-cast,
    so the contraction dim lands on SBUF partitions with no on-device
    transpose. fp16 runs the PE at full rate (4x over fp32) with ~4e-4
    relative error (11-bit significand, fp32 PSUM accumulation).
  - matmul1: h^T[f, t'] += w1[h_k, f_m].T @ xT[h_k, t']
  - gelu (tanh approx, matching jax.nn.gelu) fused with +b1 on ScalarE
  - matmul2: y[t', hh] += h^T[f_k, t'_m].T @ w2[f_k, hh], plus a rank-1
    ones.T @ b2 accumulation for the bias
  - combine: out[t', hh] = wvec[t'] * y[t', hh] on VectorE
"""

import numpy as np

import concourse.mybir as mybir
import concourse.tile as tile
from concourse import bacc
from concourse.bass_utils import run_bass_kernel_spmd

# Problem shapes (hardcoded per contract).
B, S, H, F, E, K = 2, 2048, 1024, 4096, 8, 2
T = B * S  # 4096 tokens

N_CORES = 8
PSA_BUFS = 3
PSB_BUFS = 3
XT_BUFS = 2
OUT_BUFS = 4
CAP_SPARSE = 1024
CHUNKS_SPARSE = (512, 512)
CHUNKS_MID = (512, 512, 256)
CHUNKS_DENSE = (512,) * 8

F_TILES = F // 128  # 32
H_TILES = H // 128  # 8

f16 = mybir.dt.float16
f32 = mybir.dt.float32


def _build_nc(chunks, loop_n: int = 0, with_b2: bool = True):
    """Build the per-core Bass module for sum(chunks) gathered tokens.

    loop_n is a benchmarking-only knob (repeat the body in a hardware
    For_i loop). with_b2=False omits the rank-1 b2 accumulation (exact when
    b2 is all zeros, as in the reference inputs); kernel() picks per input.
    """
    cap = sum(chunks)
    nc = bacc.Bacc(None, target_bir_lowering=False)

    xT_d = nc.dram_tensor("xT", [H, cap], f16, kind="ExternalInput")
    w1_d = nc.dram_tensor("w1", [H, F], f16, kind="ExternalInput")
    w2_d = nc.dram_tensor("w2", [F, H], f16, kind="ExternalInput")
    b1_d = nc.dram_tensor("b1T", [128, F_TILES], f32, kind="ExternalInput")
    b2_d = nc.dram_tensor("b2r", [1, H], f16, kind="ExternalInput")
    wv_d = nc.dram_tensor("wvec", [128, cap // 128], f32, kind="ExternalInput")
    out_d = nc.dram_tensor("out", [cap, H], f32, kind="ExternalOutput")

    with tile.TileContext(nc) as tc:
        with (
            tc.tile_pool(name="const", bufs=1) as constp,
            tc.tile_pool(name="xt", bufs=XT_BUFS) as xtp,
            tc.tile_pool(name="h", bufs=1) as hp,
            tc.tile_pool(name="outsb", bufs=OUT_BUFS) as outp,
            tc.tile_pool(name="psA", bufs=PSA_BUFS, space="PSUM") as psA,
            tc.tile_pool(name="psB", bufs=PSB_BUFS, space="PSUM") as psB,
        ):
            # ---- resident weights / constants ----
            # w1 loads split into column groups ordered by first use, so the
            # first phase-A groups unblock after ~1/8 of the 8MB load instead
            # of waiting for all of it (matmuls wait on per-DMA semaphores).
            w1_sb = constp.tile([128, H_TILES * F], f16)
            for fg in range(8):
                for hk in range(H_TILES):
                    nc.sync.dma_start(
                        w1_sb[:, hk * F + fg * 512 : hk * F + (fg + 1) * 512],
                        w1_d[hk * 128 : (hk + 1) * 128, fg * 512 : (fg + 1) * 512],
                    )
            w2_sb = constp.tile([128, F_TILES * H], f16)
            for fk in range(F_TILES):
                nc.sync.dma_start(
                    w2_sb[:, fk * H : (fk + 1) * H],
                    w2_d[fk * 128 : (fk + 1) * 128, :],
                )
            b1_sb = constp.tile([128, F_TILES], f32)
            nc.sync.dma_start(b1_sb[:], b1_d[:])
            b2_sb = constp.tile([1, H], f16)
            nc.sync.dma_start(b2_sb[:], b2_d[:])
            ones_sb = constp.tile([1, 128], f16)
            nc.vector.memset(ones_sb[:], 1.0)
            wvec_sb = constp.tile([128, cap // 128], f32)
            nc.sync.dma_start(wvec_sb[:], wv_d[:])

            def emit_body():
                off = 0
                for tc_sz in chunks:
                    # stream this chunk of gathered xT
                    xt_sb = xtp.tile([128, H_TILES * tc_sz], f16, name="xt_sb")
                    for hk in range(H_TILES):
                        nc.sync.dma_start(
                            xt_sb[:, hk * tc_sz : (hk + 1) * tc_sz],
                            xT_d[hk * 128 : (hk + 1) * 128, off : off + tc_sz],
                        )

                    # phase A: h^T = gelu(w1.T @ xT + b1), fp16
                    h_sb = hp.tile([128, F_TILES * tc_sz], f16, name="h_sb")
                    for fm in range(F_TILES):
                        ps = psA.tile([128, tc_sz], f32, name="psa")
                        for hk in range(H_TILES):
                            nc.tensor.matmul(
                                ps[:],
                                w1_sb[:, hk * F + fm * 128 : hk * F + (fm + 1) * 128],
                                xt_sb[:, hk * tc_sz : (hk + 1) * tc_sz],
                                start=(hk == 0),
                                stop=(hk == H_TILES - 1),
                            )
                        nc.scalar.activation(
                            h_sb[:, fm * tc_sz : (fm + 1) * tc_sz],
                            ps[:],
                            mybir.ActivationFunctionType.Gelu_apprx_tanh,
                            bias=b1_sb[:, fm : fm + 1],
                        )

                    # phase B: y = h^T.T @ w2 (+ ones.T @ b2), scale, store
                    for tm in range(tc_sz // 128):
                        wcol = (off + tm * 128) // 128
                        for nn in range(H // 512):
                            ps = psB.tile([128, 512], f32, name="psb")
                            for fk in range(F_TILES):
                                nc.tensor.matmul(
                                    ps[:],
                                    h_sb[
                                        :,
                                        fk * tc_sz + tm * 128 : fk * tc_sz
                                        + (tm + 1) * 128,
                                    ],
                                    w2_sb[
                                        :, fk * H + nn * 512 : fk * H + (nn + 1) * 512
                                    ],
                                    start=(fk == 0),
                                    stop=(not with_b2 and fk == F_TILES - 1),
                                )
                            if with_b2:
                                nc.tensor.matmul(
                                    ps[:],
                                    ones_sb[:, :],
                                    b2_sb[:, nn * 512 : (nn + 1) * 512],
                                    start=False,
                                    stop=True,
                                )
                            o_sb = outp.tile([128, 512], f32, name="o_sb")
                            nc.vector.tensor_scalar_mul(
                                o_sb[:], ps[:], wvec_sb[:, wcol : wcol + 1]
                            )
                            nc.sync.dma_start(
                                out_d[
                                    off + tm * 128 : off + (tm + 1) * 128,
                                    nn * 512 : (nn + 1) * 512,
                                ],
                                o_sb[:],
                            )
                    off += tc_sz

            if loop_n:
                import os as _os
                kw = {}
                if _os.environ.get("BENCH_STAGGER"):
                    kw["staggered_reset"] = True
                if _os.environ.get("BENCH_HINT"):
                    kw["hint_engines"] = (mybir.EngineType.PE,)
                with tc.For_i(0, loop_n, 1, **kw):
                    emit_body()
            else:
                emit_body()

    nc.compile()
    return nc


_NC_CACHE = {}


def _get_nc(chunks=CHUNKS_SPARSE, with_b2=True):
    key = (chunks, with_b2)
    if key not in _NC_CACHE:
        _NC_CACHE[key] = _build_nc(chunks, with_b2=with_b2)
    return _NC_CACHE[key]


def _route(probs, experts):
    """Per-core routed token indices and combine weights.

    Returns (idx_list, w_list): token indices (unique, sorted) routed to
    each expert and the summed probs for those tokens.
    """
    pf = np.asarray(probs, dtype=np.float32).reshape(K, T)
    ef = np.asarray(experts).reshape(K, T)
    idx_list, w_list = [], []
    for c in range(N_CORES):
        m = ef == c  # [K, T]
        sel = m.any(axis=0)
        idx = np.nonzero(sel)[0]
        w = (pf * m).sum(axis=0)[idx]
        idx_list.append(idx)
        w_list.append(w.astype(np.float32))
    return idx_list, w_list


def _prep_in_maps(x, probs, experts, w1, b1, w2, b2, cap=CAP_SPARSE, route=None):
    x = np.asarray(x, dtype=np.float32).reshape(T, H)
    xT = np.ascontiguousarray(x.T).astype(np.float16)          # [H, T]
    w1f = np.asarray(w1, dtype=np.float32).astype(np.float16)  # [E, H, F]
    w2f = np.asarray(w2, dtype=np.float32).astype(np.float16)  # [E, F, H]
    b1f = np.asarray(b1, dtype=np.float32)                     # [E, F]
    b2f = np.asarray(b2, dtype=np.float32).astype(np.float16)  # [E, H]
    if route is None:
        route = _route(probs, experts)
    idx_list, w_list = route

    in_maps = []
    for c in range(N_CORES):
        idx, w = idx_list[c], w_list[c]
        n = len(idx)
        xsel = np.zeros((H, cap), dtype=np.float16)
        xsel[:, :n] = xT[:, idx]
        wv = np.zeros(cap, dtype=np.float32)
        wv[:n] = w
        in_maps.append(
            {
                "xT": xsel,
                "w1": np.ascontiguousarray(w1f[c]),
                "w2": np.ascontiguousarray(w2f[c]),
                "b1T": np.ascontiguousarray(b1f[c].reshape(F // 128, 128).T),
                "b2r": b2f[c].reshape(1, H),
                "wvec": np.ascontiguousarray(wv.reshape(cap // 128, 128).T),
            }
        )
    return in_maps


def _unshard(results, route):
    idx_list, _ = route
    out = np.zeros((T, H), dtype=np.float32)
    for c in range(N_CORES):
        idx = idx_list[c]
        out[idx] += results[c]["out"][: len(idx)]
    return out.reshape(B, S, H)


def kernel(x, probs, experts, w1, b1, w2, b2):
    route = _route(probs, experts)
    max_n = max(len(i) for i in route[0])
    # capacity ladder: the reference distribution peaks at ~992 routed
    # tokens per expert (mean 960, sigma 27); larger variants exist only as
    # correctness fallbacks for other input distributions.
    if max_n <= CAP_SPARSE:
        chunks = CHUNKS_SPARSE
    elif max_n <= sum(CHUNKS_MID):
        chunks = CHUNKS_MID
    else:
        chunks = CHUNKS_DENSE
    nc = _get_nc(chunks, with_b2=bool(np.any(np.asarray(b2))))
    in_maps = _prep_in_maps(
        x, probs, experts, w1, b1, w2, b2, cap=sum(chunks), route=route
    )
    res = run_bass_kernel_spmd(nc, in_maps, core_ids=list(range(N_CORES)))
    return _unshard(res.results, route)



# revision 2
# speedup vs baseline: 28.0115x; 28.0115x over previous
"""Dense-MoE FFN kernel for TRN2, expert-parallel over 8 NeuronCores with
sparse token dispatch.

Reference computation (token t, hidden H=1024, ffn F=4096, E=8 experts,
top-K=2 routing):
    y_e = gelu_tanh(x @ w1_e + b1_e) @ w2_e + b2_e     (reference runs dense)
    weight[t, e] = sum_k probs[k, t] * (experts[k, t] == e)
    out[t] = sum_e weight[t, e] * y_e[t]

Tokens with weight[t, e] == 0 contribute exactly 0 to the sum, so each
expert only needs its routed tokens (~T*K/E plus dedupe, ~960 of 4096).

Sharding: expert-parallel. Core c holds expert c's weights. The host routes:
it gathers core c's tokens (x columns + combine weights) into a fixed
capacity-1024 buffer, the device computes
    out_sel[t'] = weight[t'] * (gelu_tanh(x_sel[t'] @ w1 + b1) @ w2 + b2)
and the host scatter-adds the partials into the full output (the unshard /
"all-reduce" step of the masked sum). If routing ever exceeds capacity,
larger variants of the same kernel keep the result correct for any input
distribution.

Performance structure (the kernel is PE-bound at ~220 us of fp16 matmul):
  - All weights/tokens are pre-arranged on the host into 1 MiB contiguous
    DRAM blocks so each load is ONE descriptor-cheap DMA; issue order is
    consumption order (xt chunk0, w1 groups, consts, xt chunk1, w2 groups)
    so the PE never waits on a load after the first group.
  - A short burst of warmup matmuls on zeroed tiles covers the first-DMA
    latency and brings the PE out of its cold p-state before real work.
  - matmul1: h^T[f, t'] += w1[h_k, f_m].T @ xT[h_k, t']   (fp16)
  - gelu (tanh approx, matching jax.nn.gelu) fused with +b1 on ScalarE
  - matmul2: y[t', hh] += h^T[f_k, t'_m].T @ w2[f_k, hh] (+ ones.T @ b2)
  - combine: out[t', hh] = wvec[t'] * y[t', hh] on VectorE, one 512 KiB
    store per 128-token block.
"""

import numpy as np

import concourse.mybir as mybir
import concourse.tile as tile
from concourse import bacc
from concourse.bass_utils import run_bass_kernel_spmd

# Problem shapes (hardcoded per contract).
B, S, H, F, E, K = 2, 2048, 1024, 4096, 8, 2
T = B * S  # 4096 tokens

N_CORES = 8
PSA_BUFS = 4
PSB_BUFS = 4
OUT_BUFS = 3
WARMUP_MMS = 16
CAP_SPARSE = 1024
CHUNKS_SPARSE = (512, 480)
CHUNKS_MID = (512, 512, 512)
CHUNKS_DENSE = (512,) * 8

F_TILES = F // 128  # 32
H_TILES = H // 128  # 8
WG = 8              # w1/w2 load groups

f16 = mybir.dt.float16
f32 = mybir.dt.float32


def _build_nc(chunks, loop_n: int = 0, with_b2: bool = True):
    """Build the per-core Bass module for sum(chunks) gathered tokens.

    loop_n is a benchmarking-only knob (repeat the body in a hardware
    For_i loop). with_b2=False omits the rank-1 b2 accumulation (exact when
    b2 is all zeros, as in the reference inputs); kernel() picks per input.

    DRAM layouts (host pre-arranged, see _prep_in_maps):
      xT:  [n_chunks, 128, H_TILES * tc_sz]  elem [p, hk, c] = x[off+c, hk*128+p]
      w1:  [WG, 128, H_TILES * 512]          elem [g, p, hk, c] = w1[hk*128+p, g*512+c]
      w2:  [WG, 128, (F_TILES//WG) * H]      elem [g, p, k, c] = w2[(g*4+k)*128+p, c]
    """
    cap = sum(chunks)
    n_chunks = len(chunks)
    nwv = (cap + 127) // 128
    nc = bacc.Bacc(None, target_bir_lowering=False)

    assert all(c % 128 == 0 or i == n_chunks - 1 for i, c in enumerate(chunks))
    assert all(c % 32 == 0 and c <= 512 for c in chunks)
    # xT: [128, H_TILES*cap]; chunk ci occupies cols [H_TILES*off, H_TILES*(off+tc))
    # with elem [p, hk*tc + c] = x[off+c, hk*128+p]
    xT_d = nc.dram_tensor("xT", [128, H_TILES * cap], f16,
                          kind="ExternalInput")
    w1_d = nc.dram_tensor("w1", [WG, 128, H_TILES * 512], f16,
                          kind="ExternalInput")
    w2_d = nc.dram_tensor("w2", [WG, 128, (F_TILES // WG) * H], f16,
                          kind="ExternalInput")
    b1_d = nc.dram_tensor("b1T", [128, F_TILES], f32, kind="ExternalInput")
    b2_d = nc.dram_tensor("b2r", [1, H], f16, kind="ExternalInput")
    wv_d = nc.dram_tensor("wvec", [128, nwv], f32, kind="ExternalInput")
    out_d = nc.dram_tensor("out", [cap, H], f32, kind="ExternalOutput")

    with tile.TileContext(nc) as tc:
        with (
            tc.tile_pool(name="const", bufs=1) as constp,
            tc.tile_pool(name="xt", bufs=n_chunks if not loop_n else 2) as xtp,
            tc.tile_pool(name="h", bufs=1) as hp,
            tc.tile_pool(name="outsb", bufs=OUT_BUFS) as outp,
            tc.tile_pool(name="psA", bufs=PSA_BUFS, space="PSUM") as psA,
            tc.tile_pool(name="psB", bufs=PSB_BUFS, space="PSUM") as psB,
        ):
            # SBUF weight layouts mirror the DRAM group blocks:
            #   w1_sb[p, g*4096 + hk*512 + c] = w1[hk*128+p, g*512+c]
            #   w2_sb[p, (g*4+k)*H + c]       = w2[(g*4+k)*128+p, c]
            w1_sb = constp.tile([128, WG * H_TILES * 512], f16)
            w2_sb = constp.tile([128, F_TILES * H], f16)
            b1_sb = constp.tile([128, F_TILES], f32)
            b2_sb = constp.tile([1, H], f16)
            ones_sb = constp.tile([1, 128], f16)
            wvec_sb = constp.tile([128, nwv], f32)

            def w1_slice(hk, fm):
                g, m = fm // 4, fm % 4
                off = g * (H_TILES * 512) + hk * 512 + m * 128
                return w1_sb[:, off : off + 128]

            def load_w1_group(g):
                nc.sync.dma_start(
                    w1_sb[:, g * (H_TILES * 512) : (g + 1) * (H_TILES * 512)],
                    w1_d[g],
                )

            def load_w2_group(g):
                sz = (F_TILES // WG) * H
                nc.sync.dma_start(w2_sb[:, g * sz : (g + 1) * sz], w2_d[g])

            def load_small():
                nc.sync.dma_start(b1_sb[:], b1_d[:])
                nc.vector.memset(ones_sb[:], 1.0)
                nc.sync.dma_start(wvec_sb[:], wv_d[:])
                if with_b2:
                    nc.sync.dma_start(b2_sb[:], b2_d[:])

            def emit_chunk(xt_sb, off, tc_sz, last=False):
                # phase A: h^T = gelu(w1.T @ xT + b1), fp16
                h_sb = hp.tile([128, F_TILES * tc_sz], f16, name="h_sb")
                for fm in range(F_TILES):
                    ps = psA.tile([128, 512], f32, name="psa")
                    for hk in range(H_TILES):
                        nc.tensor.matmul(
                            ps[:, :tc_sz],
                            w1_slice(hk, fm),
                            xt_sb[:, hk * tc_sz : (hk + 1) * tc_sz],
                            start=(hk == 0),
                            stop=(hk == H_TILES - 1),
                        )
                    nc.scalar.activation(
                        h_sb[:, fm * tc_sz : (fm + 1) * tc_sz],
                        ps[:, :tc_sz],
                        mybir.ActivationFunctionType.Gelu_apprx_tanh,
                        bias=b1_sb[:, fm : fm + 1],
                    )

                # phase B: y = h^T.T @ w2 (+ ones.T @ b2), scale, store
                n_tm = (tc_sz + 127) // 128
                for tm in range(n_tm):
                    t0 = tm * 128
                    tsz = min(128, tc_sz - t0)
                    wcol = (off + t0) // 128
                    final = last and tm == n_tm - 1
                    o_sb = outp.tile([128, H], f32, name="o_sb")
                    for nn in range(H // 512):
                        ps = psB.tile([128, 512], f32, name="psb")
                        for fk in range(F_TILES):
                            nc.tensor.matmul(
                                ps[:tsz, :],
                                h_sb[:, fk * tc_sz + t0 : fk * tc_sz + t0 + tsz],
                                w2_sb[
                                    :, fk * H + nn * 512 : fk * H + (nn + 1) * 512
                                ],
                                start=(fk == 0),
                                stop=(not with_b2 and fk == F_TILES - 1),
                            )
                        if with_b2:
                            nc.tensor.matmul(
                                ps[:tsz, :],
                                ones_sb[:, :tsz],
                                b2_sb[:, nn * 512 : (nn + 1) * 512],
                                start=False,
                                stop=True,
                            )
                        nc.vector.tensor_scalar_mul(
                            o_sb[:tsz, nn * 512 : (nn + 1) * 512],
                            ps[:tsz, :],
                            wvec_sb[:tsz, wcol : wcol + 1],
                        )
                        if final:
                            # split the very last store so the nn=0 half
                            # drains while nn=1's matmuls still run
                            nc.sync.dma_start(
                                out_d[
                                    off + t0 : off + t0 + tsz,
                                    nn * 512 : (nn + 1) * 512,
                                ],
                                o_sb[:tsz, nn * 512 : (nn + 1) * 512],
                            )
                    if not final:
                        nc.sync.dma_start(
                            out_d[off + t0 : off + t0 + tsz, :], o_sb[:tsz, :]
                        )

            if loop_n:
                # benchmark mode: weights resident, xT streamed per iteration
                for g in range(WG):
                    load_w1_group(g)
                for g in range(WG):
                    load_w2_group(g)
                load_small()
                import os as _os
                kw = {}
                if _os.environ.get("BENCH_STAGGER"):
                    kw["staggered_reset"] = True
                if _os.environ.get("BENCH_HINT"):
                    kw["hint_engines"] = (mybir.EngineType.PE,)
                with tc.For_i(0, loop_n, 1, **kw):
                    off = 0
                    for ci, tc_sz in enumerate(chunks):
                        xt_sb = xtp.tile([128, H_TILES * tc_sz], f16, name="xt_sb")
                        nc.sync.dma_start(
                            xt_sb[:],
                            xT_d[:, H_TILES * off : H_TILES * (off + tc_sz)],
                        )
                        emit_chunk(xt_sb, off, tc_sz, last=(ci == n_chunks - 1))
                        off += tc_sz
            else:
                # single-shot: DMA issue order IS the critical path.
                # Warmup matmuls on zeroed tiles keep the PE busy (and its
                # p-state ramping) while the first loads land.
                warm_src = constp.tile([128, 512 + 128], f16)
                nc.vector.memset(warm_src[:], 0.0)
                warm_ps = psA.tile([128, 512], f32, name="psa")
                xt_tiles = []
                offs = []
                off = 0
                for ci, tc_sz in enumerate(chunks):
                    xt_tiles.append(
                        xtp.tile([128, H_TILES * tc_sz], f16, name=f"xt{ci}")
                    )
                    offs.append(off)
                    off += tc_sz

                def load_xt(ci):
                    o, tc_sz = offs[ci], chunks[ci]
                    nc.sync.dma_start(
                        xt_tiles[ci][:],
                        xT_d[:, H_TILES * o : H_TILES * (o + tc_sz)],
                    )

                load_xt(0)
                load_w1_group(0)
                load_small()
                for w in range(WARMUP_MMS):
                    nc.tensor.matmul(
                        warm_ps[:],
                        warm_src[:, 512:640],
                        warm_src[:, 0:512],
                        start=True,
                        stop=True,
                    )
                for g in range(1, WG):
                    load_w1_group(g)
                for ci in range(1, n_chunks):
                    load_xt(ci)
                for g in range(WG):
                    load_w2_group(g)
                for ci, tc_sz in enumerate(chunks):
                    emit_chunk(
                        xt_tiles[ci], offs[ci], tc_sz, last=(ci == n_chunks - 1)
                    )

    nc.compile()
    return nc


_NC_CACHE = {}


def _get_nc(chunks=CHUNKS_SPARSE, with_b2=True):
    key = (chunks, with_b2)
    if key not in _NC_CACHE:
        _NC_CACHE[key] = _build_nc(chunks, with_b2=with_b2)
    return _NC_CACHE[key]


def _route(probs, experts):
    """Per-core routed token indices and combine weights.

    Returns (idx_list, w_list): token indices (unique, sorted) routed to
    each expert and the summed probs for those tokens.
    """
    pf = np.asarray(probs, dtype=np.float32).reshape(K, T)
    ef = np.asarray(experts).reshape(K, T)
    idx_list, w_list = [], []
    for c in range(N_CORES):
        m = ef == c  # [K, T]
        sel = m.any(axis=0)
        idx = np.nonzero(sel)[0]
        w = (pf * m).sum(axis=0)[idx]
        idx_list.append(idx)
        w_list.append(w.astype(np.float32))
    return idx_list, w_list


def _prep_in_maps(x, probs, experts, w1, b1, w2, b2, chunks=CHUNKS_SPARSE,
                  route=None):
    cap = sum(chunks)
    nwv = (cap + 127) // 128
    x = np.asarray(x, dtype=np.float32).reshape(T, H)
    xT = np.ascontiguousarray(x.T).astype(np.float16)          # [H, T]
    w1f = np.asarray(w1, dtype=np.float32).astype(np.float16)  # [E, H, F]
    w2f = np.asarray(w2, dtype=np.float32).astype(np.float16)  # [E, F, H]
    b1f = np.asarray(b1, dtype=np.float32)                     # [E, F]
    b2f = np.asarray(b2, dtype=np.float32).astype(np.float16)  # [E, H]
    if route is None:
        route = _route(probs, experts)
    idx_list, w_list = route

    in_maps = []
    for c in range(N_CORES):
        idx, w = idx_list[c], w_list[c]
        n = len(idx)
        xsel = np.zeros((H, cap), dtype=np.float16)
        xsel[:, :n] = xT[:, idx]
        # per chunk [128p, hk, tc] <- xsel[hk*128+p, off+c], concat on free dim
        blocks = []
        off = 0
        for tc_sz in chunks:
            blk = (
                xsel[:, off : off + tc_sz]
                .reshape(H_TILES, 128, tc_sz)
                .transpose(1, 0, 2)
                .reshape(128, H_TILES * tc_sz)
            )
            blocks.append(blk)
            off += tc_sz
        xdr = np.ascontiguousarray(np.concatenate(blocks, axis=1))
        # [g, 128p, hk, 512c] <- w1[hk*128+p, g*512+c]
        w1dr = np.ascontiguousarray(
            w1f[c].reshape(H_TILES, 128, WG, 512)
            .transpose(2, 1, 0, 3)
            .reshape(WG, 128, H_TILES * 512)
        )
        # [g, 128p, k, 1024c] <- w2[(g*4+k)*128+p, c]
        kpg = F_TILES // WG
        w2dr = np.ascontiguousarray(
            w2f[c].reshape(WG, kpg, 128, H)
            .transpose(0, 2, 1, 3)
            .reshape(WG, 128, kpg * H)
        )
        wv = np.zeros(nwv * 128, dtype=np.float32)
        wv[:n] = w
        in_maps.append(
            {
                "xT": xdr,
                "w1": w1dr,
                "w2": w2dr,
                "b1T": np.ascontiguousarray(b1f[c].reshape(F // 128, 128).T),
                "b2r": b2f[c].reshape(1, H),
                "wvec": np.ascontiguousarray(wv.reshape(nwv, 128).T),
            }
        )
    return in_maps


def _unshard(results, route):
    idx_list, _ = route
    out = np.zeros((T, H), dtype=np.float32)
    for c in range(N_CORES):
        idx = idx_list[c]
        out[idx] += results[c]["out"][: len(idx)]
    return out.reshape(B, S, H)


def _pick_chunks(max_n):
    """Smallest 32-granular capacity >= max routed count, as <=512 chunks.

    The reference distribution peaks at ~992 routed tokens per expert (mean
    960, sigma 27) -> (512, 480). Anything else still compiles a correct
    variant (compile cached per chunk tuple)."""
    capq = max(128, ((max_n + 31) // 32) * 32)
    full, rem = divmod(capq, 512)
    return (512,) * full + ((rem,) if rem else ())


def kernel(x, probs, experts, w1, b1, w2, b2):
    route = _route(probs, experts)
    max_n = max(len(i) for i in route[0])
    chunks = _pick_chunks(max_n)
    nc = _get_nc(chunks, with_b2=bool(np.any(np.asarray(b2))))
    in_maps = _prep_in_maps(
        x, probs, experts, w1, b1, w2, b2, chunks=chunks, route=route
    )
    res = run_bass_kernel_spmd(nc, in_maps, core_ids=list(range(N_CORES)))
    return _unshard(res.results, route)


# revision 5
# speedup vs baseline: 31.1683x; 1.1127x over previous
"""Dense-MoE FFN kernel for TRN2, expert-parallel over 8 NeuronCores with
sparse token dispatch.

Reference computation (token t, hidden H=1024, ffn F=4096, E=8 experts,
top-K=2 routing):
    y_e = gelu_tanh(x @ w1_e + b1_e) @ w2_e + b2_e     (reference runs dense)
    weight[t, e] = sum_k probs[k, t] * (experts[k, t] == e)
    out[t] = sum_e weight[t, e] * y_e[t]

Tokens with weight[t, e] == 0 contribute exactly 0 to the sum, so each
expert only needs its routed tokens (~T*K/E plus dedupe, ~960 of 4096).

Sharding: expert-parallel. Core c holds expert c's weights. The host routes:
it gathers core c's tokens (x columns + combine weights) into a fixed
capacity-1024 buffer, the device computes
    out_sel[t'] = weight[t'] * (gelu_tanh(x_sel[t'] @ w1 + b1) @ w2 + b2)
and the host scatter-adds the partials into the full output (the unshard /
"all-reduce" step of the masked sum). If routing ever exceeds capacity,
larger variants of the same kernel keep the result correct for any input
distribution.

Performance structure (the kernel is PE-bound at ~220 us of fp16 matmul):
  - All weights/tokens are pre-arranged on the host into 1 MiB contiguous
    DRAM blocks so each load is ONE descriptor-cheap DMA; issue order is
    consumption order (xt chunk0, w1 groups, consts, xt chunk1, w2 groups)
    so the PE never waits on a load after the first group.
  - A short burst of warmup matmuls on zeroed tiles covers the first-DMA
    latency and brings the PE out of its cold p-state before real work.
  - matmul1: h^T[f, t'] += w1[h_k, f_m].T @ xT[h_k, t']   (fp16)
  - gelu (tanh approx, matching jax.nn.gelu) fused with +b1 on ScalarE
  - matmul2: y[t', hh] += h^T[f_k, t'_m].T @ w2[f_k, hh] (+ ones.T @ b2)
  - combine: out[t', hh] = wvec[t'] * y[t', hh] on VectorE, one 512 KiB
    store per 128-token block.
"""

import numpy as np

import concourse.mybir as mybir
import concourse.tile as tile
from concourse import bacc
from concourse.bass_utils import run_bass_kernel_spmd

# Problem shapes (hardcoded per contract).
B, S, H, F, E, K = 2, 2048, 1024, 4096, 8, 2
T = B * S  # 4096 tokens

N_CORES = 8
PSA_BUFS = 4
PSB_BUFS = 4
OUT_BUFS = 3
WARMUP_MMS = 16
CAP_SPARSE = 1024
CHUNKS_SPARSE = (512, 480)
CHUNKS_MID = (512, 512, 512)
CHUNKS_DENSE = (512,) * 8

F_TILES = F // 128  # 32
H_TILES = H // 128  # 8
WG = 8              # w1/w2 load groups

f16 = mybir.dt.float16
f32 = mybir.dt.float32


def _build_nc(chunks, loop_n: int = 0, with_b2: bool = True):
    """Build the per-core Bass module for sum(chunks) gathered tokens.

    loop_n is a benchmarking-only knob (repeat the body in a hardware
    For_i loop). with_b2=False omits the rank-1 b2 accumulation (exact when
    b2 is all zeros, as in the reference inputs); kernel() picks per input.

    DRAM layouts (host pre-arranged, see _prep_in_maps):
      xT:  [n_chunks, 128, H_TILES * tc_sz]  elem [p, hk, c] = x[off+c, hk*128+p]
      w1:  [WG, 128, H_TILES * 512]          elem [g, p, hk, c] = w1[hk*128+p, g*512+c]
      w2:  [WG, 128, (F_TILES//WG) * H]      elem [g, p, k, c] = w2[(g*4+k)*128+p, c]
    """
    cap = sum(chunks)
    n_chunks = len(chunks)
    nwv = (cap + 127) // 128
    nc = bacc.Bacc(None, target_bir_lowering=False)

    assert all(c % 128 == 0 or i == n_chunks - 1 for i, c in enumerate(chunks))
    assert all(c % 32 == 0 and c <= 512 for c in chunks)
    # xT: [128, H_TILES*cap]; chunk ci occupies cols [H_TILES*off, H_TILES*(off+tc))
    # with elem [p, hk*tc + c] = x[off+c, hk*128+p]
    xT_d = nc.dram_tensor("xT", [128, H_TILES * cap], f16,
                          kind="ExternalInput")
    w1_d = nc.dram_tensor("w1", [WG, 128, H_TILES * 512], f16,
                          kind="ExternalInput")
    w2_d = nc.dram_tensor("w2", [WG, 128, (F_TILES // WG) * H], f16,
                          kind="ExternalInput")
    b1_d = nc.dram_tensor("b1T", [128, F_TILES], f32, kind="ExternalInput")
    b2_d = nc.dram_tensor("b2r", [1, H], f16, kind="ExternalInput")
    wv_d = nc.dram_tensor("wvec", [128, nwv], f32, kind="ExternalInput")
    out_d = nc.dram_tensor("out", [cap, H], f32, kind="ExternalOutput")

    with tile.TileContext(nc) as tc:
        with (
            tc.tile_pool(name="const", bufs=1) as constp,
            tc.tile_pool(name="xt", bufs=n_chunks if not loop_n else 2) as xtp,
            tc.tile_pool(name="h", bufs=1) as hp,
            tc.tile_pool(name="outsb", bufs=OUT_BUFS) as outp,
            tc.tile_pool(name="psA", bufs=PSA_BUFS, space="PSUM") as psA,
            tc.tile_pool(name="psB", bufs=PSB_BUFS, space="PSUM") as psB,
        ):
            # SBUF weight layouts mirror the DRAM group blocks:
            #   w1_sb[p, g*4096 + hk*512 + c] = w1[hk*128+p, g*512+c]
            #   w2_sb[p, (g*4+k)*H + c]       = w2[(g*4+k)*128+p, c]
            w1_sb = constp.tile([128, WG * H_TILES * 512], f16)
            w2_sb = constp.tile([128, F_TILES * H], f16)
            b1_sb = constp.tile([128, F_TILES], f32)
            b2_sb = constp.tile([1, H], f16)
            ones_sb = constp.tile([1, 128], f16)
            wvec_sb = constp.tile([128, nwv], f32)

            def w1_slice(hk, fm):
                g, m = fm // 4, fm % 4
                off = g * (H_TILES * 512) + hk * 512 + m * 128
                return w1_sb[:, off : off + 128]

            def load_w1_group(g):
                nc.sync.dma_start(
                    w1_sb[:, g * (H_TILES * 512) : (g + 1) * (H_TILES * 512)],
                    w1_d[g],
                )

            def load_w2_group(g):
                sz = (F_TILES // WG) * H
                nc.sync.dma_start(w2_sb[:, g * sz : (g + 1) * sz], w2_d[g])

            def load_small():
                nc.sync.dma_start(b1_sb[:], b1_d[:])
                nc.vector.memset(ones_sb[:], 1.0)
                nc.sync.dma_start(wvec_sb[:], wv_d[:])
                if with_b2:
                    nc.sync.dma_start(b2_sb[:], b2_d[:])

            def emit_chunk(xt_sb, off, tc_sz, last=False, do_a=True, do_b=True):
                # phase A: h^T = gelu(w1.T @ xT + b1), fp16
                h_sb = hp.tile([128, F_TILES * tc_sz], f16, name="h_sb")
                if not do_a:
                    pass
                for fm in range(F_TILES if do_a else 0):
                    ps = psA.tile([128, 512], f32, name="psa")
                    for hk in range(H_TILES):
                        nc.tensor.matmul(
                            ps[:, :tc_sz],
                            w1_slice(hk, fm),
                            xt_sb[:, hk * tc_sz : (hk + 1) * tc_sz],
                            start=(hk == 0),
                            stop=(hk == H_TILES - 1),
                        )
                    nc.scalar.activation(
                        h_sb[:, fm * tc_sz : (fm + 1) * tc_sz],
                        ps[:, :tc_sz],
                        mybir.ActivationFunctionType.Gelu_apprx_tanh,
                        bias=b1_sb[:, fm : fm + 1],
                    )

                # phase B: y = h^T.T @ w2 (+ ones.T @ b2), scale, store
                n_tm = (tc_sz + 127) // 128 if do_b else 0
                for tm in range(n_tm):
                    t0 = tm * 128
                    tsz = min(128, tc_sz - t0)
                    wcol = (off + t0) // 128
                    final = last and tm == n_tm - 1
                    o_sb = outp.tile([128, H], f32, name="o_sb")
                    for nn in range(H // 512):
                        ps = psB.tile([128, 512], f32, name="psb")
                        for fk in range(F_TILES):
                            nc.tensor.matmul(
                                ps[:tsz, :],
                                h_sb[:, fk * tc_sz + t0 : fk * tc_sz + t0 + tsz],
                                w2_sb[
                                    :, fk * H + nn * 512 : fk * H + (nn + 1) * 512
                                ],
                                start=(fk == 0),
                                stop=(not with_b2 and fk == F_TILES - 1),
                            )
                        if with_b2:
                            nc.tensor.matmul(
                                ps[:tsz, :],
                                ones_sb[:, :tsz],
                                b2_sb[:, nn * 512 : (nn + 1) * 512],
                                start=False,
                                stop=True,
                            )
                        nc.vector.tensor_scalar_mul(
                            o_sb[:tsz, nn * 512 : (nn + 1) * 512],
                            ps[:tsz, :],
                            wvec_sb[:tsz, wcol : wcol + 1],
                        )
                        if final:
                            # split the very last store so the nn=0 half
                            # drains while nn=1's matmuls still run
                            nc.sync.dma_start(
                                out_d[
                                    off + t0 : off + t0 + tsz,
                                    nn * 512 : (nn + 1) * 512,
                                ],
                                o_sb[:tsz, nn * 512 : (nn + 1) * 512],
                            )
                    if not final:
                        nc.sync.dma_start(
                            out_d[off + t0 : off + t0 + tsz, :], o_sb[:tsz, :]
                        )

            if loop_n:
                # benchmark mode: weights resident, xT streamed per iteration
                for g in range(WG):
                    load_w1_group(g)
                for g in range(WG):
                    load_w2_group(g)
                load_small()
                import os as _os
                kw = {}
                if _os.environ.get("BENCH_STAGGER"):
                    kw["staggered_reset"] = True
                if _os.environ.get("BENCH_HINT"):
                    kw["hint_engines"] = (mybir.EngineType.PE,)
                do_a = not _os.environ.get("BENCH_BONLY")
                do_b = not _os.environ.get("BENCH_AONLY")
                with tc.For_i(0, loop_n, 1, **kw):
                    off = 0
                    for ci, tc_sz in enumerate(chunks):
                        xt_sb = xtp.tile([128, H_TILES * tc_sz], f16, name="xt_sb")
                        nc.sync.dma_start(
                            xt_sb[:],
                            xT_d[:, H_TILES * off : H_TILES * (off + tc_sz)],
                        )
                        emit_chunk(xt_sb, off, tc_sz, last=(ci == n_chunks - 1),
                                   do_a=do_a, do_b=do_b)
                        off += tc_sz
            else:
                # single-shot: DMA issue order IS the critical path.
                # Warmup matmuls on zeroed tiles keep the PE busy (and its
                # p-state ramping) while the first loads land.
                warm_src = constp.tile([128, 512 + 128], f16)
                nc.vector.memset(warm_src[:], 0.0)
                warm_ps = psA.tile([128, 512], f32, name="psa")
                xt_tiles = []
                offs = []
                off = 0
                for ci, tc_sz in enumerate(chunks):
                    xt_tiles.append(
                        xtp.tile([128, H_TILES * tc_sz], f16, name=f"xt{ci}")
                    )
                    offs.append(off)
                    off += tc_sz

                def load_xt(ci):
                    o, tc_sz = offs[ci], chunks[ci]
                    nc.sync.dma_start(
                        xt_tiles[ci][:],
                        xT_d[:, H_TILES * o : H_TILES * (o + tc_sz)],
                    )

                load_xt(0)
                load_w1_group(0)
                load_small()
                for w in range(WARMUP_MMS):
                    nc.tensor.matmul(
                        warm_ps[:],
                        warm_src[:, 512:640],
                        warm_src[:, 0:512],
                        start=True,
                        stop=True,
                    )
                for g in range(1, WG):
                    load_w1_group(g)
                for ci in range(1, n_chunks):
                    load_xt(ci)
                for g in range(WG):
                    load_w2_group(g)
                for ci, tc_sz in enumerate(chunks):
                    emit_chunk(
                        xt_tiles[ci], offs[ci], tc_sz, last=(ci == n_chunks - 1)
                    )

    nc.compile()
    return nc


_NC_CACHE = {}


def _get_nc(chunks=CHUNKS_SPARSE, with_b2=True):
    key = (chunks, with_b2)
    if key not in _NC_CACHE:
        _NC_CACHE[key] = _build_nc(chunks, with_b2=with_b2)
    return _NC_CACHE[key]


def _route(probs, experts):
    """Per-core routed token indices and combine weights.

    Returns (idx_list, w_list): token indices (unique, sorted) routed to
    each expert and the summed probs for those tokens.
    """
    pf = np.asarray(probs, dtype=np.float32).reshape(K, T)
    ef = np.asarray(experts).reshape(K, T)
    idx_list, w_list = [], []
    for c in range(N_CORES):
        m = ef == c  # [K, T]
        sel = m.any(axis=0)
        idx = np.nonzero(sel)[0]
        w = (pf * m).sum(axis=0)[idx]
        idx_list.append(idx)
        w_list.append(w.astype(np.float32))
    return idx_list, w_list


def _prep_in_maps(x, probs, experts, w1, b1, w2, b2, chunks=CHUNKS_SPARSE,
                  route=None):
    cap = sum(chunks)
    nwv = (cap + 127) // 128
    x = np.asarray(x, dtype=np.float32).reshape(T, H)
    xT = np.ascontiguousarray(x.T).astype(np.float16)          # [H, T]
    w1f = np.asarray(w1, dtype=np.float32).astype(np.float16)  # [E, H, F]
    w2f = np.asarray(w2, dtype=np.float32).astype(np.float16)  # [E, F, H]
    b1f = np.asarray(b1, dtype=np.float32)                     # [E, F]
    b2f = np.asarray(b2, dtype=np.float32).astype(np.float16)  # [E, H]
    if route is None:
        route = _route(probs, experts)
    idx_list, w_list = route

    in_maps = []
    for c in range(N_CORES):
        idx, w = idx_list[c], w_list[c]
        n = len(idx)
        xsel = np.zeros((H, cap), dtype=np.float16)
        xsel[:, :n] = xT[:, idx]
        # per chunk [128p, hk, tc] <- xsel[hk*128+p, off+c], concat on free dim
        blocks = []
        off = 0
        for tc_sz in chunks:
            blk = (
                xsel[:, off : off + tc_sz]
                .reshape(H_TILES, 128, tc_sz)
                .transpose(1, 0, 2)
                .reshape(128, H_TILES * tc_sz)
            )
            blocks.append(blk)
            off += tc_sz
        xdr = np.ascontiguousarray(np.concatenate(blocks, axis=1))
        # [g, 128p, hk, 512c] <- w1[hk*128+p, g*512+c]
        w1dr = np.ascontiguousarray(
            w1f[c].reshape(H_TILES, 128, WG, 512)
            .transpose(2, 1, 0, 3)
            .reshape(WG, 128, H_TILES * 512)
        )
        # [g, 128p, k, 1024c] <- w2[(g*4+k)*128+p, c]
        kpg = F_TILES // WG
        w2dr = np.ascontiguousarray(
            w2f[c].reshape(WG, kpg, 128, H)
            .transpose(0, 2, 1, 3)
            .reshape(WG, 128, kpg * H)
        )
        wv = np.zeros(nwv * 128, dtype=np.float32)
        wv[:n] = w
        in_maps.append(
            {
                "xT": xdr,
                "w1": w1dr,
                "w2": w2dr,
                "b1T": np.ascontiguousarray(b1f[c].reshape(F // 128, 128).T),
                "b2r": b2f[c].reshape(1, H),
                "wvec": np.ascontiguousarray(wv.reshape(nwv, 128).T),
            }
        )
    return in_maps


def _unshard(results, route):
    idx_list, _ = route
    out = np.zeros((T, H), dtype=np.float32)
    for c in range(N_CORES):
        idx = idx_list[c]
        out[idx] += results[c]["out"][: len(idx)]
    return out.reshape(B, S, H)


def _pick_chunks(max_n):
    """Smallest 32-granular capacity >= max routed count, as <=512 chunks.

    The reference distribution peaks at ~992 routed tokens per expert (mean
    960, sigma 27) -> (512, 480). Anything else still compiles a correct
    variant (compile cached per chunk tuple)."""
    capq = max(128, ((max_n + 31) // 32) * 32)
    full, rem = divmod(capq, 512)
    return (512,) * full + ((rem,) if rem else ())


def kernel(x, probs, experts, w1, b1, w2, b2):
    route = _route(probs, experts)
    max_n = max(len(i) for i in route[0])
    chunks = _pick_chunks(max_n)
    nc = _get_nc(chunks, with_b2=bool(np.any(np.asarray(b2))))
    in_maps = _prep_in_maps(
        x, probs, experts, w1, b1, w2, b2, chunks=chunks, route=route
    )
    res = run_bass_kernel_spmd(nc, in_maps, core_ids=list(range(N_CORES)))
    return _unshard(res.results, route)


# revision 8
# speedup vs baseline: 31.6324x; 1.0149x over previous
"""Dense-MoE FFN kernel for TRN2, expert-parallel over 8 NeuronCores with
sparse token dispatch.

Reference computation (token t, hidden H=1024, ffn F=4096, E=8 experts,
top-K=2 routing):
    y_e = gelu_tanh(x @ w1_e + b1_e) @ w2_e + b2_e     (reference runs dense)
    weight[t, e] = sum_k probs[k, t] * (experts[k, t] == e)
    out[t] = sum_e weight[t, e] * y_e[t]

Tokens with weight[t, e] == 0 contribute exactly 0 to the sum, so each
expert only needs its routed tokens (~T*K/E plus dedupe, ~960 of 4096).

Sharding: expert-parallel. Core c holds expert c's weights. The host routes:
it gathers core c's tokens (x columns + combine weights) into a fixed
capacity-1024 buffer, the device computes
    out_sel[t'] = weight[t'] * (gelu_tanh(x_sel[t'] @ w1 + b1) @ w2 + b2)
and the host scatter-adds the partials into the full output (the unshard /
"all-reduce" step of the masked sum). If routing ever exceeds capacity,
larger variants of the same kernel keep the result correct for any input
distribution.

Performance structure (the kernel is PE-bound at ~220 us of fp16 matmul):
  - All weights/tokens are pre-arranged on the host into 1 MiB contiguous
    DRAM blocks so each load is ONE descriptor-cheap DMA; issue order is
    consumption order (xt chunk0, w1 groups, consts, xt chunk1, w2 groups)
    so the PE never waits on a load after the first group.
  - A short burst of warmup matmuls on zeroed tiles covers the first-DMA
    latency and brings the PE out of its cold p-state before real work.
  - matmul1: h^T[f, t'] += w1[h_k, f_m].T @ xT[h_k, t']   (fp16)
  - gelu (tanh approx, matching jax.nn.gelu) fused with +b1 on ScalarE
  - matmul2: y[t', hh] += h^T[f_k, t'_m].T @ w2[f_k, hh] (+ ones.T @ b2)
  - combine: out[t', hh] = wvec[t'] * y[t', hh] on VectorE, one 512 KiB
    store per 128-token block.
"""

import numpy as np

import concourse.mybir as mybir
import concourse.tile as tile
from concourse import bacc
from concourse.bass_utils import run_bass_kernel_spmd

# Problem shapes (hardcoded per contract).
B, S, H, F, E, K = 2, 2048, 1024, 4096, 8, 2
T = B * S  # 4096 tokens

N_CORES = 8
PSA_BUFS = 4
PSB_BUFS = 4
OUT_BUFS = 3
WARMUP_MMS = 10
CAP_SPARSE = 1024
CHUNKS_SPARSE = (512, 480)
CHUNKS_MID = (512, 512, 512)
CHUNKS_DENSE = (512,) * 8

F_TILES = F // 128  # 32
H_TILES = H // 128  # 8
WG = 8              # w1/w2 load groups

f16 = mybir.dt.float16
f32 = mybir.dt.float32


def _build_nc(chunks, loop_n: int = 0, with_b2: bool = True):
    """Build the per-core Bass module for sum(chunks) gathered tokens.

    loop_n is a benchmarking-only knob (repeat the body in a hardware
    For_i loop). with_b2=False omits the rank-1 b2 accumulation (exact when
    b2 is all zeros, as in the reference inputs); kernel() picks per input.

    DRAM layouts (host pre-arranged, see _prep_in_maps):
      xT:  [n_chunks, 128, H_TILES * tc_sz]  elem [p, hk, c] = x[off+c, hk*128+p]
      w1:  [WG, 128, H_TILES * 512]          elem [g, p, hk, c] = w1[hk*128+p, g*512+c]
      w2:  [WG, 128, (F_TILES//WG) * H]      elem [g, p, k, c] = w2[(g*4+k)*128+p, c]
    """
    cap = sum(chunks)
    n_chunks = len(chunks)
    nwv = (cap + 127) // 128
    nc = bacc.Bacc(None, target_bir_lowering=False)

    assert all(c % 128 == 0 or i == n_chunks - 1 for i, c in enumerate(chunks))
    assert all(c % 32 == 0 and c <= 512 for c in chunks)
    # xT: [128, H_TILES*cap]; chunk ci occupies cols [H_TILES*off, H_TILES*(off+tc))
    # with elem [p, hk*tc + c] = x[off+c, hk*128+p]
    xT_d = nc.dram_tensor("xT", [128, H_TILES * cap], f16,
                          kind="ExternalInput")
    w1_d = nc.dram_tensor("w1", [WG, 128, H_TILES * 512], f16,
                          kind="ExternalInput")
    w2_d = nc.dram_tensor("w2", [WG, 128, (F_TILES // WG) * H], f16,
                          kind="ExternalInput")
    b1_d = nc.dram_tensor("b1T", [128, F_TILES], f32, kind="ExternalInput")
    b2_d = nc.dram_tensor("b2r", [1, H], f16, kind="ExternalInput")
    # combine weights replicated across partitions (DVE lanes cannot
    # broadcast across partitions, so the host materializes the row)
    wv_d = nc.dram_tensor("wvec", [128, nwv * 128], f32, kind="ExternalInput")
    # output is y^T [H, cap]: phase B computes token-moving matmuls so its
    # cost scales with the actual token count; the host transposes back
    out_d = nc.dram_tensor("out", [H, cap], f32, kind="ExternalOutput")

    with tile.TileContext(nc) as tc:
        with (
            tc.tile_pool(name="const", bufs=1) as constp,
            tc.tile_pool(name="xt", bufs=n_chunks if not loop_n else 2) as xtp,
            tc.tile_pool(name="h", bufs=1) as hp,
            tc.tile_pool(name="outsb", bufs=OUT_BUFS) as outp,
            tc.tile_pool(name="psA", bufs=PSA_BUFS, space="PSUM") as psA,
            tc.tile_pool(name="psB", bufs=PSB_BUFS, space="PSUM") as psB,
        ):
            # SBUF weight layouts mirror the DRAM group blocks:
            #   w1_sb[p, g*4096 + hk*512 + c] = w1[hk*128+p, g*512+c]
            #   w2_sb[p, (g*4+k)*H + c]       = w2[(g*4+k)*128+p, c]
            w1_sb = constp.tile([128, WG * H_TILES * 512], f16)
            w2_sb = constp.tile([128, F_TILES * H], f16)
            b1_sb = constp.tile([128, F_TILES], f32)
            b2_sb = constp.tile([1, H], f16)
            ones_sb = constp.tile([1, 512], f16)
            wvec_sb = constp.tile([128, nwv * 128], f32)

            def w1_slice(hk, fm):
                g, m = fm // 4, fm % 4
                off = g * (H_TILES * 512) + hk * 512 + m * 128
                return w1_sb[:, off : off + 128]

            def load_w1_group(g):
                nc.sync.dma_start(
                    w1_sb[:, g * (H_TILES * 512) : (g + 1) * (H_TILES * 512)],
                    w1_d[g],
                )

            def load_w2_group(g):
                sz = (F_TILES // WG) * H
                nc.sync.dma_start(w2_sb[:, g * sz : (g + 1) * sz], w2_d[g])

            def load_small():
                nc.sync.dma_start(b1_sb[:], b1_d[:])
                nc.vector.memset(ones_sb[:], 1.0)
                nc.sync.dma_start(wvec_sb[:], wv_d[:])
                if with_b2:
                    nc.sync.dma_start(b2_sb[:], b2_d[:])

            def emit_chunk(xt_sb, off, tc_sz, last=False, do_a=True, do_b=True,
                           split_head=False):
                # phase A: h^T = gelu(w1.T @ xT + b1), fp16.
                # split_head: the first 4 psum groups issue their hk 0-3
                # matmuls first, so the PE starts on the first half-MB of
                # xT/w1 while the second half is still in flight.
                h_sb = hp.tile([128, F_TILES * tc_sz], f16, name="h_sb")

                def mm_a(ps, fm, hk):
                    nc.tensor.matmul(
                        ps[:, :tc_sz],
                        w1_slice(hk, fm),
                        xt_sb[:, hk * tc_sz : (hk + 1) * tc_sz],
                        start=(hk == 0),
                        stop=(hk == H_TILES - 1),
                    )

                def act_a(ps, fm):
                    nc.scalar.activation(
                        h_sb[:, fm * tc_sz : (fm + 1) * tc_sz],
                        ps[:, :tc_sz],
                        mybir.ActivationFunctionType.Gelu_apprx_tanh,
                        bias=b1_sb[:, fm : fm + 1],
                    )

                head = min(PSA_BUFS, 4) if (do_a and split_head) else 0
                head_tiles = [psA.tile([128, 512], f32, name="psa")
                              for _ in range(head)]
                for fm in range(head):
                    for hk in range(H_TILES // 2):
                        mm_a(head_tiles[fm], fm, hk)
                for fm in range(head):
                    for hk in range(H_TILES // 2, H_TILES):
                        mm_a(head_tiles[fm], fm, hk)
                    act_a(head_tiles[fm], fm)
                for fm in range(head, F_TILES if do_a else 0):
                    ps = psA.tile([128, 512], f32, name="psa")
                    for hk in range(H_TILES):
                        mm_a(ps, fm, hk)
                    act_a(ps, fm)

                # phase B: y^T[hh, t] = w2[f, hh].T @ h^T[f, t] (+ b2.T @ 1),
                # tokens are the moving operand so the matmul cost scales
                # with the routed token count; combine on DVE, store y^T.
                for hm in range(H_TILES if do_b else 0):
                    ps = psB.tile([128, 512], f32, name="psb")
                    for fk in range(F_TILES):
                        nc.tensor.matmul(
                            ps[:, :tc_sz],
                            w2_sb[:, fk * H + hm * 128 : fk * H + (hm + 1) * 128],
                            h_sb[:, fk * tc_sz : (fk + 1) * tc_sz],
                            start=(fk == 0),
                            stop=(not with_b2 and fk == F_TILES - 1),
                        )
                    if with_b2:
                        nc.tensor.matmul(
                            ps[:, :tc_sz],
                            b2_sb[:, hm * 128 : (hm + 1) * 128],
                            ones_sb[:, :tc_sz],
                            start=False,
                            stop=True,
                        )
                    o_sb = outp.tile([128, 512], f32, name="o_sb")
                    if last and hm == H_TILES - 1:
                        # split the very last combine+store so the first half
                        # drains while the second is still being scaled
                        hh = tc_sz // 2
                        for lo, hi in ((0, hh), (hh, tc_sz)):
                            nc.vector.tensor_mul(
                                o_sb[:, lo:hi],
                                ps[:, lo:hi],
                                wvec_sb[:, off + lo : off + hi],
                            )
                            nc.sync.dma_start(
                                out_d[hm * 128 : (hm + 1) * 128,
                                      off + lo : off + hi],
                                o_sb[:, lo:hi],
                            )
                    else:
                        nc.vector.tensor_mul(
                            o_sb[:, :tc_sz],
                            ps[:, :tc_sz],
                            wvec_sb[:, off : off + tc_sz],
                        )
                        nc.sync.dma_start(
                            out_d[hm * 128 : (hm + 1) * 128, off : off + tc_sz],
                            o_sb[:, :tc_sz],
                        )

            if loop_n:
                # benchmark mode: weights resident, xT streamed per iteration
                for g in range(WG):
                    load_w1_group(g)
                for g in range(WG):
                    load_w2_group(g)
                load_small()
                import os as _os
                kw = {}
                if _os.environ.get("BENCH_STAGGER"):
                    kw["staggered_reset"] = True
                if _os.environ.get("BENCH_HINT"):
                    kw["hint_engines"] = (mybir.EngineType.PE,)
                do_a = not _os.environ.get("BENCH_BONLY")
                do_b = not _os.environ.get("BENCH_AONLY")
                pure = _os.environ.get("BENCH_PURE")  # "1": fixed stationary,
                # "2": rotating stationary. Pure back-to-back MM rate probe.
                with tc.For_i(0, loop_n, 1, **kw):
                    if pure:
                        for i in range(1024):
                            ws = (i % 8) * 128 if pure == "2" else 0
                            ps = psA.tile([128, 512], f32, name="psa")
                            nc.tensor.matmul(
                                ps[:],
                                w1_sb[:, ws : ws + 128],
                                w1_sb[:, 1024:1536],
                                start=True,
                                stop=True,
                            )
                        continue_body = None  # noqa: F841
                    else:
                        off = 0
                        for ci, tc_sz in enumerate(chunks):
                            xt_sb = xtp.tile(
                                [128, H_TILES * tc_sz], f16, name="xt_sb"
                            )
                            nc.sync.dma_start(
                                xt_sb[:],
                                xT_d[:, H_TILES * off : H_TILES * (off + tc_sz)],
                            )
                            emit_chunk(
                                xt_sb, off, tc_sz, last=(ci == n_chunks - 1),
                                do_a=do_a, do_b=do_b,
                            )
                            off += tc_sz
            else:
                # single-shot: DMA issue order IS the critical path.
                # Warmup matmuls on zeroed tiles keep the PE busy (and its
                # p-state ramping) while the first loads land.
                warm_src = constp.tile([128, 512 + 128], f16)
                nc.vector.memset(warm_src[:], 0.0)
                warm_ps = psA.tile([128, 512], f32, name="psa")
                xt_tiles = []
                offs = []
                off = 0
                for ci, tc_sz in enumerate(chunks):
                    xt_tiles.append(
                        xtp.tile([128, H_TILES * tc_sz], f16, name=f"xt{ci}")
                    )
                    offs.append(off)
                    off += tc_sz

                def load_xt(ci):
                    o, tc_sz = offs[ci], chunks[ci]
                    nc.sync.dma_start(
                        xt_tiles[ci][:],
                        xT_d[:, H_TILES * o : H_TILES * (o + tc_sz)],
                    )

                # first chunk's xT and w1 group 0 arrive as interleaved
                # halves (hk 0-3, then hk 4-7) so the first phase-A matmuls
                # start ~3.5 us earlier (see split_head)
                half_x = (H_TILES // 2) * chunks[0]
                half_w = (H_TILES // 2) * 512
                nc.sync.dma_start(
                    xt_tiles[0][:, :half_x], xT_d[:, :half_x]
                )
                nc.sync.dma_start(
                    w1_sb[:, :half_w], w1_d[0][:, :half_w]
                )
                nc.sync.dma_start(
                    xt_tiles[0][:, half_x : 2 * half_x],
                    xT_d[:, half_x : 2 * half_x],
                )
                nc.sync.dma_start(
                    w1_sb[:, half_w : 2 * half_w],
                    w1_d[0][:, half_w : 2 * half_w],
                )
                load_small()
                for w in range(WARMUP_MMS):
                    nc.tensor.matmul(
                        warm_ps[:],
                        warm_src[:, 512:640],
                        warm_src[:, 0:512],
                        start=True,
                        stop=True,
                    )
                for g in range(1, WG):
                    load_w1_group(g)
                for ci in range(1, n_chunks):
                    load_xt(ci)
                for g in range(WG):
                    load_w2_group(g)
                for ci, tc_sz in enumerate(chunks):
                    emit_chunk(
                        xt_tiles[ci], offs[ci], tc_sz,
                        last=(ci == n_chunks - 1), split_head=(ci == 0),
                    )

    nc.compile()
    return nc


_NC_CACHE = {}


def _get_nc(chunks=CHUNKS_SPARSE, with_b2=True):
    key = (chunks, with_b2)
    if key not in _NC_CACHE:
        _NC_CACHE[key] = _build_nc(chunks, with_b2=with_b2)
    return _NC_CACHE[key]


def _route(probs, experts):
    """Per-core routed token indices and combine weights.

    Returns (idx_list, w_list): token indices (unique, sorted) routed to
    each expert and the summed probs for those tokens.
    """
    pf = np.asarray(probs, dtype=np.float32).reshape(K, T)
    ef = np.asarray(experts).reshape(K, T)
    idx_list, w_list = [], []
    for c in range(N_CORES):
        m = ef == c  # [K, T]
        sel = m.any(axis=0)
        idx = np.nonzero(sel)[0]
        w = (pf * m).sum(axis=0)[idx]
        idx_list.append(idx)
        w_list.append(w.astype(np.float32))
    return idx_list, w_list


def _prep_in_maps(x, probs, experts, w1, b1, w2, b2, chunks=CHUNKS_SPARSE,
                  route=None):
    cap = sum(chunks)
    nwv = (cap + 127) // 128
    x = np.asarray(x, dtype=np.float32).reshape(T, H)
    xT = np.ascontiguousarray(x.T).astype(np.float16)          # [H, T]
    w1f = np.asarray(w1, dtype=np.float32).astype(np.float16)  # [E, H, F]
    w2f = np.asarray(w2, dtype=np.float32).astype(np.float16)  # [E, F, H]
    b1f = np.asarray(b1, dtype=np.float32)                     # [E, F]
    b2f = np.asarray(b2, dtype=np.float32).astype(np.float16)  # [E, H]
    if route is None:
        route = _route(probs, experts)
    idx_list, w_list = route

    in_maps = []
    for c in range(N_CORES):
        idx, w = idx_list[c], w_list[c]
        n = len(idx)
        xsel = np.zeros((H, cap), dtype=np.float16)
        xsel[:, :n] = xT[:, idx]
        # per chunk [128p, hk, tc] <- xsel[hk*128+p, off+c], concat on free dim
        blocks = []
        off = 0
        for tc_sz in chunks:
            blk = (
                xsel[:, off : off + tc_sz]
                .reshape(H_TILES, 128, tc_sz)
                .transpose(1, 0, 2)
                .reshape(128, H_TILES * tc_sz)
            )
            blocks.append(blk)
            off += tc_sz
        xdr = np.ascontiguousarray(np.concatenate(blocks, axis=1))
        # [g, 128p, hk, 512c] <- w1[hk*128+p, g*512+c]
        w1dr = np.ascontiguousarray(
            w1f[c].reshape(H_TILES, 128, WG, 512)
            .transpose(2, 1, 0, 3)
            .reshape(WG, 128, H_TILES * 512)
        )
        # [g, 128p, k, 1024c] <- w2[(g*4+k)*128+p, c]
        kpg = F_TILES // WG
        w2dr = np.ascontiguousarray(
            w2f[c].reshape(WG, kpg, 128, H)
            .transpose(0, 2, 1, 3)
            .reshape(WG, 128, kpg * H)
        )
        wv = np.zeros(nwv * 128, dtype=np.float32)
        wv[:n] = w
        in_maps.append(
            {
                "xT": xdr,
                "w1": w1dr,
                "w2": w2dr,
                "b1T": np.ascontiguousarray(b1f[c].reshape(F // 128, 128).T),
                "b2r": b2f[c].reshape(1, H),
                # replicated across partitions for the DVE combine
                "wvec": np.ascontiguousarray(
                    np.broadcast_to(wv, (128, nwv * 128))
                ),
            }
        )
    return in_maps


def _unshard(results, route):
    idx_list, _ = route
    out = np.zeros((T, H), dtype=np.float32)
    for c in range(N_CORES):
        idx = idx_list[c]
        # device output is y^T [H, cap]
        out[idx] += results[c]["out"][:, : len(idx)].T
    return out.reshape(B, S, H)


def _pick_chunks(max_n):
    """Smallest 32-granular capacity >= max routed count, as <=512 chunks.

    The reference distribution peaks at ~992 routed tokens per expert (mean
    960, sigma 27) -> (512, 480). Anything else still compiles a correct
    variant (compile cached per chunk tuple)."""
    capq = max(128, ((max_n + 31) // 32) * 32)
    full, rem = divmod(capq, 512)
    return (512,) * full + ((rem,) if rem else ())


def kernel(x, probs, experts, w1, b1, w2, b2):
    route = _route(probs, experts)
    max_n = max(len(i) for i in route[0])
    chunks = _pick_chunks(max_n)
    nc = _get_nc(chunks, with_b2=bool(np.any(np.asarray(b2))))
    in_maps = _prep_in_maps(
        x, probs, experts, w1, b1, w2, b2, chunks=chunks, route=route
    )
    res = run_bass_kernel_spmd(nc, in_maps, core_ids=list(range(N_CORES)))
    return _unshard(res.results, route)


# revision 15
# speedup vs baseline: 31.6818x; 1.0016x over previous
"""Dense-MoE FFN kernel for TRN2, expert-parallel over 8 NeuronCores with
sparse token dispatch.

Reference computation (token t, hidden H=1024, ffn F=4096, E=8 experts,
top-K=2 routing):
    y_e = gelu_tanh(x @ w1_e + b1_e) @ w2_e + b2_e     (reference runs dense)
    weight[t, e] = sum_k probs[k, t] * (experts[k, t] == e)
    out[t] = sum_e weight[t, e] * y_e[t]

Tokens with weight[t, e] == 0 contribute exactly 0 to the sum, so each
expert only needs its routed tokens (~T*K/E plus dedupe, ~960 of 4096).

Sharding: expert-parallel. Core c holds expert c's weights. The host routes:
it gathers core c's tokens (x columns + combine weights) into a fixed
capacity-1024 buffer, the device computes
    out_sel[t'] = weight[t'] * (gelu_tanh(x_sel[t'] @ w1 + b1) @ w2 + b2)
and the host scatter-adds the partials into the full output (the unshard /
"all-reduce" step of the masked sum). If routing ever exceeds capacity,
larger variants of the same kernel keep the result correct for any input
distribution.

Performance structure (the kernel is PE-bound at ~220 us of fp16 matmul):
  - All weights/tokens are pre-arranged on the host into 1 MiB contiguous
    DRAM blocks so each load is ONE descriptor-cheap DMA; issue order is
    consumption order (xt chunk0, w1 groups, consts, xt chunk1, w2 groups)
    so the PE never waits on a load after the first group.
  - A short burst of warmup matmuls on zeroed tiles covers the first-DMA
    latency and brings the PE out of its cold p-state before real work.
  - matmul1: h^T[f, t'] += w1[h_k, f_m].T @ xT[h_k, t']   (fp16)
  - gelu (tanh approx, matching jax.nn.gelu) fused with +b1 on ScalarE
  - matmul2: y[t', hh] += h^T[f_k, t'_m].T @ w2[f_k, hh] (+ ones.T @ b2)
  - combine: out[t', hh] = wvec[t'] * y[t', hh] on VectorE, one 512 KiB
    store per 128-token block.
"""

import numpy as np

import concourse.mybir as mybir
import concourse.tile as tile
from concourse import bacc
from concourse.bass_utils import run_bass_kernel_spmd

# Problem shapes (hardcoded per contract).
B, S, H, F, E, K = 2, 2048, 1024, 4096, 8, 2
T = B * S  # 4096 tokens

N_CORES = 8
PSA_BUFS = 4
PSB_BUFS = 4
OUT_BUFS = 3
WARMUP_MMS = 10
CAP_SPARSE = 1024
CHUNKS_SPARSE = (512, 480)
CHUNKS_MID = (512, 512, 512)
CHUNKS_DENSE = (512,) * 8

F_TILES = F // 128  # 32
H_TILES = H // 128  # 8
WG = 8              # w1/w2 load groups

f16 = mybir.dt.float16
f32 = mybir.dt.float32


def _build_nc(chunks, loop_n: int = 0, with_b2: bool = True):
    """Build the per-core Bass module for sum(chunks) gathered tokens.

    loop_n is a benchmarking-only knob (repeat the body in a hardware
    For_i loop). with_b2=False omits the rank-1 b2 accumulation (exact when
    b2 is all zeros, as in the reference inputs); kernel() picks per input.

    DRAM layouts (host pre-arranged, see _prep_in_maps):
      xT:  [n_chunks, 128, H_TILES * tc_sz]  elem [p, hk, c] = x[off+c, hk*128+p]
      w1:  [WG, 128, H_TILES * 512]          elem [g, p, hk, c] = w1[hk*128+p, g*512+c]
      w2:  [WG, 128, (F_TILES//WG) * H]      elem [g, p, k, c] = w2[(g*4+k)*128+p, c]
    """
    cap = sum(chunks)
    n_chunks = len(chunks)
    nwv = (cap + 127) // 128
    nc = bacc.Bacc(None, target_bir_lowering=False)

    assert all(c % 128 == 0 or i == n_chunks - 1 for i, c in enumerate(chunks))
    assert all(c % 32 == 0 and c <= 512 for c in chunks)
    # xT: [128, H_TILES*cap]; chunk ci occupies cols [H_TILES*off, H_TILES*(off+tc))
    # with elem [p, hk*tc + c] = x[off+c, hk*128+p]
    xT_d = nc.dram_tensor("xT", [128, H_TILES * cap], f16,
                          kind="ExternalInput")
    w1_d = nc.dram_tensor("w1", [WG, 128, H_TILES * 512], f16,
                          kind="ExternalInput")
    w2_d = nc.dram_tensor("w2", [WG, 128, (F_TILES // WG) * H], f16,
                          kind="ExternalInput")
    b1_d = nc.dram_tensor("b1T", [128, F_TILES], f32, kind="ExternalInput")
    b2_d = nc.dram_tensor("b2r", [1, H], f16, kind="ExternalInput")
    # combine weights replicated across partitions (DVE lanes cannot
    # broadcast across partitions, so the host materializes the row)
    wv_d = nc.dram_tensor("wvec", [128, nwv * 128], f32, kind="ExternalInput")
    # output is y^T [H, cap]: phase B computes token-moving matmuls so its
    # cost scales with the actual token count; the host transposes back
    out_d = nc.dram_tensor("out", [H, cap], f32, kind="ExternalOutput")

    with tile.TileContext(nc) as tc:
        with (
            tc.tile_pool(name="const", bufs=1) as constp,
            tc.tile_pool(name="xt", bufs=n_chunks if not loop_n else 2) as xtp,
            tc.tile_pool(name="h", bufs=1) as hp,
            tc.tile_pool(name="outsb", bufs=OUT_BUFS) as outp,
            tc.tile_pool(name="psA", bufs=PSA_BUFS, space="PSUM") as psA,
            tc.tile_pool(name="psB", bufs=PSB_BUFS, space="PSUM") as psB,
        ):
            # SBUF weight layouts mirror the DRAM group blocks:
            #   w1_sb[p, g*4096 + hk*512 + c] = w1[hk*128+p, g*512+c]
            #   w2_sb[p, (g*4+k)*H + c]       = w2[(g*4+k)*128+p, c]
            w1_sb = constp.tile([128, WG * H_TILES * 512], f16)
            w2_sb = constp.tile([128, F_TILES * H], f16)
            b1_sb = constp.tile([128, F_TILES], f32)
            b2_sb = constp.tile([1, H], f16)
            ones_sb = constp.tile([1, 512], f16)
            wvec_sb = constp.tile([128, nwv * 128], f32)

            def w1_slice(hk, fm):
                g, m = fm // 4, fm % 4
                off = g * (H_TILES * 512) + hk * 512 + m * 128
                return w1_sb[:, off : off + 128]

            def load_w1_group(g):
                nc.sync.dma_start(
                    w1_sb[:, g * (H_TILES * 512) : (g + 1) * (H_TILES * 512)],
                    w1_d[g],
                )

            def load_w2_group(g):
                sz = (F_TILES // WG) * H
                nc.sync.dma_start(w2_sb[:, g * sz : (g + 1) * sz], w2_d[g])

            def load_small():
                nc.sync.dma_start(b1_sb[:], b1_d[:])
                nc.vector.memset(ones_sb[:], 1.0)
                nc.sync.dma_start(wvec_sb[:], wv_d[:])
                if with_b2:
                    nc.sync.dma_start(b2_sb[:], b2_d[:])

            def emit_chunk(xt_sb, off, tc_sz, last=False, do_a=True, do_b=True,
                           split_head=False):
                # phase A: h^T = gelu(w1.T @ xT + b1), fp16.
                # split_head: the first 4 psum groups issue their hk 0-3
                # matmuls first, so the PE starts on the first half-MB of
                # xT/w1 while the second half is still in flight.
                h_sb = hp.tile([128, F_TILES * tc_sz], f16, name="h_sb")

                def mm_a(ps, fm, hk):
                    nc.tensor.matmul(
                        ps[:, :tc_sz],
                        w1_slice(hk, fm),
                        xt_sb[:, hk * tc_sz : (hk + 1) * tc_sz],
                        start=(hk == 0),
                        stop=(hk == H_TILES - 1),
                    )

                def act_a(ps, fm):
                    nc.scalar.activation(
                        h_sb[:, fm * tc_sz : (fm + 1) * tc_sz],
                        ps[:, :tc_sz],
                        mybir.ActivationFunctionType.Gelu_apprx_tanh,
                        bias=b1_sb[:, fm : fm + 1],
                    )

                head = min(PSA_BUFS, 4) if (do_a and split_head) else 0
                head_tiles = [psA.tile([128, 512], f32, name="psa")
                              for _ in range(head)]
                for fm in range(head):
                    for hk in range(H_TILES // 2):
                        mm_a(head_tiles[fm], fm, hk)
                for fm in range(head):
                    for hk in range(H_TILES // 2, H_TILES):
                        mm_a(head_tiles[fm], fm, hk)
                    act_a(head_tiles[fm], fm)
                for fm in range(head, F_TILES if do_a else 0):
                    ps = psA.tile([128, 512], f32, name="psa")
                    for hk in range(H_TILES):
                        mm_a(ps, fm, hk)
                    act_a(ps, fm)

                # phase B: y^T[hh, t] = w2[f, hh].T @ h^T[f, t] (+ b2.T @ 1),
                # tokens are the moving operand so the matmul cost scales
                # with the routed token count; combine on DVE, store y^T.
                for hm in range(H_TILES if do_b else 0):
                    ps = psB.tile([128, 512], f32, name="psb")
                    for fk in range(F_TILES):
                        nc.tensor.matmul(
                            ps[:, :tc_sz],
                            w2_sb[:, fk * H + hm * 128 : fk * H + (hm + 1) * 128],
                            h_sb[:, fk * tc_sz : (fk + 1) * tc_sz],
                            start=(fk == 0),
                            stop=(not with_b2 and fk == F_TILES - 1),
                        )
                    if with_b2:
                        nc.tensor.matmul(
                            ps[:, :tc_sz],
                            b2_sb[:, hm * 128 : (hm + 1) * 128],
                            ones_sb[:, :tc_sz],
                            start=False,
                            stop=True,
                        )
                    o_sb = outp.tile([128, 512], f32, name="o_sb")
                    if last and hm == H_TILES - 1:
                        # split the very last combine+store so the first half
                        # drains while the second is still being scaled
                        hh = tc_sz // 2
                        for lo, hi in ((0, hh), (hh, tc_sz)):
                            nc.vector.tensor_mul(
                                o_sb[:, lo:hi],
                                ps[:, lo:hi],
                                wvec_sb[:, off + lo : off + hi],
                            )
                            nc.sync.dma_start(
                                out_d[hm * 128 : (hm + 1) * 128,
                                      off + lo : off + hi],
                                o_sb[:, lo:hi],
                            )
                    else:
                        nc.vector.tensor_mul(
                            o_sb[:, :tc_sz],
                            ps[:, :tc_sz],
                            wvec_sb[:, off : off + tc_sz],
                        )
                        nc.sync.dma_start(
                            out_d[hm * 128 : (hm + 1) * 128, off : off + tc_sz],
                            o_sb[:, :tc_sz],
                        )

            if loop_n:
                # benchmark mode: weights resident, xT streamed per iteration
                for g in range(WG):
                    load_w1_group(g)
                for g in range(WG):
                    load_w2_group(g)
                load_small()
                import os as _os
                kw = {}
                if _os.environ.get("BENCH_STAGGER"):
                    kw["staggered_reset"] = True
                if _os.environ.get("BENCH_HINT"):
                    kw["hint_engines"] = (mybir.EngineType.PE,)
                do_a = not _os.environ.get("BENCH_BONLY")
                do_b = not _os.environ.get("BENCH_AONLY")
                pure = _os.environ.get("BENCH_PURE")  # "1": fixed stationary,
                # "2": rotating stationary. Pure back-to-back MM rate probe.
                with tc.For_i(0, loop_n, 1, **kw):
                    if pure:
                        for i in range(1024):
                            ws = (i % 8) * 128 if pure == "2" else 0
                            ps = psA.tile([128, 512], f32, name="psa")
                            nc.tensor.matmul(
                                ps[:],
                                w1_sb[:, ws : ws + 128],
                                w1_sb[:, 1024:1536],
                                start=True,
                                stop=True,
                            )
                        continue_body = None  # noqa: F841
                    else:
                        off = 0
                        for ci, tc_sz in enumerate(chunks):
                            xt_sb = xtp.tile(
                                [128, H_TILES * tc_sz], f16, name="xt_sb"
                            )
                            nc.sync.dma_start(
                                xt_sb[:],
                                xT_d[:, H_TILES * off : H_TILES * (off + tc_sz)],
                            )
                            emit_chunk(
                                xt_sb, off, tc_sz, last=(ci == n_chunks - 1),
                                do_a=do_a, do_b=do_b,
                            )
                            off += tc_sz
            else:
                # single-shot: DMA issue order IS the critical path.
                # Warmup matmuls on zeroed tiles keep the PE busy (and its
                # p-state ramping) while the first loads land.
                warm_src = constp.tile([128, 512 + 128], f16)
                nc.vector.memset(warm_src[:], 0.0)
                warm_ps = psA.tile([128, 512], f32, name="psa")
                xt_tiles = []
                offs = []
                off = 0
                for ci, tc_sz in enumerate(chunks):
                    xt_tiles.append(
                        xtp.tile([128, H_TILES * tc_sz], f16, name=f"xt{ci}")
                    )
                    offs.append(off)
                    off += tc_sz

                def load_xt(ci):
                    o, tc_sz = offs[ci], chunks[ci]
                    nc.sync.dma_start(
                        xt_tiles[ci][:],
                        xT_d[:, H_TILES * o : H_TILES * (o + tc_sz)],
                    )

                # first chunk's xT and w1 group 0 arrive as interleaved
                # halves (hk 0-3, then hk 4-7) so the first phase-A matmuls
                # start ~3.5 us earlier (see split_head)
                half_x = (H_TILES // 2) * chunks[0]
                half_w = (H_TILES // 2) * 512
                nc.sync.dma_start(
                    xt_tiles[0][:, :half_x], xT_d[:, :half_x]
                )
                nc.sync.dma_start(
                    w1_sb[:, :half_w], w1_d[0][:, :half_w]
                )
                nc.sync.dma_start(
                    xt_tiles[0][:, half_x : 2 * half_x],
                    xT_d[:, half_x : 2 * half_x],
                )
                nc.sync.dma_start(
                    w1_sb[:, half_w : 2 * half_w],
                    w1_d[0][:, half_w : 2 * half_w],
                )
                load_small()
                for w in range(WARMUP_MMS):
                    nc.tensor.matmul(
                        warm_ps[:],
                        warm_src[:, 512:640],
                        warm_src[:, 0:512],
                        start=True,
                        stop=True,
                    )
                for g in range(1, WG):
                    load_w1_group(g)
                for ci in range(1, n_chunks):
                    load_xt(ci)
                for g in range(WG):
                    load_w2_group(g)
                for ci, tc_sz in enumerate(chunks):
                    emit_chunk(
                        xt_tiles[ci], offs[ci], tc_sz,
                        last=(ci == n_chunks - 1), split_head=(ci == 0),
                    )

    nc.compile()
    return nc


_NC_CACHE = {}


def _get_nc(chunks=CHUNKS_SPARSE, with_b2=True):
    key = (chunks, with_b2)
    if key not in _NC_CACHE:
        _NC_CACHE[key] = _build_nc(chunks, with_b2=with_b2)
    return _NC_CACHE[key]


def _route(probs, experts):
    """Per-core routed token indices and combine weights.

    Returns (idx_list, w_list): token indices (unique, sorted) routed to
    each expert and the summed probs for those tokens.
    """
    pf = np.asarray(probs, dtype=np.float32).reshape(K, T)
    ef = np.asarray(experts).reshape(K, T)
    idx_list, w_list = [], []
    for c in range(N_CORES):
        m = ef == c  # [K, T]
        sel = m.any(axis=0)
        idx = np.nonzero(sel)[0]
        w = (pf * m).sum(axis=0)[idx]
        idx_list.append(idx)
        w_list.append(w.astype(np.float32))
    return idx_list, w_list


def _prep_in_maps(x, probs, experts, w1, b1, w2, b2, chunks=CHUNKS_SPARSE,
                  route=None):
    cap = sum(chunks)
    nwv = (cap + 127) // 128
    x = np.asarray(x, dtype=np.float32).reshape(T, H)
    xT = np.ascontiguousarray(x.T).astype(np.float16)          # [H, T]
    w1f = np.asarray(w1, dtype=np.float32).astype(np.float16)  # [E, H, F]
    w2f = np.asarray(w2, dtype=np.float32).astype(np.float16)  # [E, F, H]
    b1f = np.asarray(b1, dtype=np.float32)                     # [E, F]
    b2f = np.asarray(b2, dtype=np.float32).astype(np.float16)  # [E, H]
    if route is None:
        route = _route(probs, experts)
    idx_list, w_list = route

    in_maps = []
    for c in range(N_CORES):
        idx, w = idx_list[c], w_list[c]
        n = len(idx)
        xsel = np.zeros((H, cap), dtype=np.float16)
        xsel[:, :n] = xT[:, idx]
        # per chunk [128p, hk, tc] <- xsel[hk*128+p, off+c], concat on free dim
        blocks = []
        off = 0
        for tc_sz in chunks:
            blk = (
                xsel[:, off : off + tc_sz]
                .reshape(H_TILES, 128, tc_sz)
                .transpose(1, 0, 2)
                .reshape(128, H_TILES * tc_sz)
            )
            blocks.append(blk)
            off += tc_sz
        xdr = np.ascontiguousarray(np.concatenate(blocks, axis=1))
        # [g, 128p, hk, 512c] <- w1[hk*128+p, g*512+c]
        w1dr = np.ascontiguousarray(
            w1f[c].reshape(H_TILES, 128, WG, 512)
            .transpose(2, 1, 0, 3)
            .reshape(WG, 128, H_TILES * 512)
        )
        # [g, 128p, k, 1024c] <- w2[(g*4+k)*128+p, c]
        kpg = F_TILES // WG
        w2dr = np.ascontiguousarray(
            w2f[c].reshape(WG, kpg, 128, H)
            .transpose(0, 2, 1, 3)
            .reshape(WG, 128, kpg * H)
        )
        wv = np.zeros(nwv * 128, dtype=np.float32)
        wv[:n] = w
        in_maps.append(
            {
                "xT": xdr,
                "w1": w1dr,
                "w2": w2dr,
                "b1T": np.ascontiguousarray(b1f[c].reshape(F // 128, 128).T),
                "b2r": b2f[c].reshape(1, H),
                # replicated across partitions for the DVE combine
                "wvec": np.ascontiguousarray(
                    np.broadcast_to(wv, (128, nwv * 128))
                ),
            }
        )
    return in_maps


def _unshard(results, route):
    idx_list, _ = route
    out = np.zeros((T, H), dtype=np.float32)
    for c in range(N_CORES):
        idx = idx_list[c]
        # device output is y^T [H, cap]
        out[idx] += results[c]["out"][:, : len(idx)].T
    return out.reshape(B, S, H)


def _pick_chunks(max_n):
    """Smallest 32-granular capacity >= max routed count, as <=512 chunks.

    The reference distribution peaks at ~992 routed tokens per expert (mean
    960, sigma 27) -> (512, 480). Anything else still compiles a correct
    variant (compile cached per chunk tuple)."""
    capq = max(128, ((max_n + 31) // 32) * 32)
    full, rem = divmod(capq, 512)
    return (512,) * full + ((rem,) if rem else ())


def kernel(x, probs, experts, w1, b1, w2, b2):
    route = _route(probs, experts)
    max_n = max(len(i) for i in route[0])
    chunks = _pick_chunks(max_n)
    nc = _get_nc(chunks, with_b2=bool(np.any(np.asarray(b2))))
    in_maps = _prep_in_maps(
        x, probs, experts, w1, b1, w2, b2, chunks=chunks, route=route
    )
    res = run_bass_kernel_spmd(nc, in_maps, core_ids=list(range(N_CORES)))
    return _unshard(res.results, route)


# revision 20
# speedup vs baseline: 31.7880x; 1.0034x over previous
"""Dense-MoE FFN kernel for TRN2, expert-parallel over 8 NeuronCores with
sparse token dispatch.

Reference computation (token t, hidden H=1024, ffn F=4096, E=8 experts,
top-K=2 routing):
    y_e = gelu_tanh(x @ w1_e + b1_e) @ w2_e + b2_e     (reference runs dense)
    weight[t, e] = sum_k probs[k, t] * (experts[k, t] == e)
    out[t] = sum_e weight[t, e] * y_e[t]

Tokens with weight[t, e] == 0 contribute exactly 0 to the sum, so each
expert only needs its routed tokens (~T*K/E plus dedupe, ~960 of 4096).

Sharding: expert-parallel. Core c holds expert c's weights. The host routes:
it gathers core c's tokens (x columns + combine weights) into a fixed
capacity-1024 buffer, the device computes
    out_sel[t'] = weight[t'] * (gelu_tanh(x_sel[t'] @ w1 + b1) @ w2 + b2)
and the host scatter-adds the partials into the full output (the unshard /
"all-reduce" step of the masked sum). If routing ever exceeds capacity,
larger variants of the same kernel keep the result correct for any input
distribution.

Performance structure (the kernel is PE-bound at ~220 us of fp16 matmul):
  - All weights/tokens are pre-arranged on the host into 1 MiB contiguous
    DRAM blocks so each load is ONE descriptor-cheap DMA; issue order is
    consumption order (xt chunk0, w1 groups, consts, xt chunk1, w2 groups)
    so the PE never waits on a load after the first group.
  - A short burst of warmup matmuls on zeroed tiles covers the first-DMA
    latency and brings the PE out of its cold p-state before real work.
  - matmul1: h^T[f, t'] += w1[h_k, f_m].T @ xT[h_k, t']   (fp16)
  - gelu (tanh approx, matching jax.nn.gelu) fused with +b1 on ScalarE
  - matmul2: y[t', hh] += h^T[f_k, t'_m].T @ w2[f_k, hh] (+ ones.T @ b2)
  - combine: out[t', hh] = wvec[t'] * y[t', hh] on VectorE, one 512 KiB
    store per 128-token block.
"""

import numpy as np

import concourse.mybir as mybir
import concourse.tile as tile
from concourse import bacc
from concourse.bass_utils import run_bass_kernel_spmd

# Problem shapes (hardcoded per contract).
B, S, H, F, E, K = 2, 2048, 1024, 4096, 8, 2
T = B * S  # 4096 tokens

N_CORES = 8
PSA_BUFS = 4
PSB_BUFS = 4
OUT_BUFS = 3
WARMUP_MMS = 6
CAP_SPARSE = 1024
CHUNKS_SPARSE = (512, 480)
CHUNKS_MID = (512, 512, 512)
CHUNKS_DENSE = (512,) * 8

F_TILES = F // 128  # 32
H_TILES = H // 128  # 8
WG = 8              # w1/w2 load groups

f16 = mybir.dt.float16
f32 = mybir.dt.float32


def _build_nc(chunks, loop_n: int = 0, with_b2: bool = True):
    """Build the per-core Bass module for sum(chunks) gathered tokens.

    loop_n is a benchmarking-only knob (repeat the body in a hardware
    For_i loop). with_b2=False omits the rank-1 b2 accumulation (exact when
    b2 is all zeros, as in the reference inputs); kernel() picks per input.

    DRAM layouts (host pre-arranged, see _prep_in_maps):
      xT:  [n_chunks, 128, H_TILES * tc_sz]  elem [p, hk, c] = x[off+c, hk*128+p]
      w1:  [WG, 128, H_TILES * 512]          elem [g, p, hk, c] = w1[hk*128+p, g*512+c]
      w2:  [WG, 128, (F_TILES//WG) * H]      elem [g, p, k, c] = w2[(g*4+k)*128+p, c]
    """
    cap = sum(chunks)
    n_chunks = len(chunks)
    nwv = (cap + 127) // 128
    nc = bacc.Bacc(None, target_bir_lowering=False)

    assert all(c % 128 == 0 or i == n_chunks - 1 for i, c in enumerate(chunks))
    assert all(c % 32 == 0 and c <= 512 for c in chunks)
    # xT: [128, H_TILES*cap]; chunk ci occupies cols [H_TILES*off, H_TILES*(off+tc))
    # with elem [p, hk*tc + c] = x[off+c, hk*128+p]
    xT_d = nc.dram_tensor("xT", [128, H_TILES * cap], f16,
                          kind="ExternalInput")
    w1_d = nc.dram_tensor("w1", [WG, 128, H_TILES * 512], f16,
                          kind="ExternalInput")
    w2_d = nc.dram_tensor("w2", [WG, 128, (F_TILES // WG) * H], f16,
                          kind="ExternalInput")
    b1_d = nc.dram_tensor("b1T", [128, F_TILES], f32, kind="ExternalInput")
    b2_d = nc.dram_tensor("b2r", [1, H], f16, kind="ExternalInput")
    # combine weights replicated across partitions (DVE lanes cannot
    # broadcast across partitions, so the host materializes the row)
    wv_d = nc.dram_tensor("wvec", [128, nwv * 128], f32, kind="ExternalInput")
    # output is y^T [H, cap]: phase B computes token-moving matmuls so its
    # cost scales with the actual token count; the host transposes back
    out_d = nc.dram_tensor("out", [H, cap], f32, kind="ExternalOutput")

    with tile.TileContext(nc) as tc:
        with (
            tc.tile_pool(name="const", bufs=1) as constp,
            tc.tile_pool(name="xt", bufs=n_chunks if not loop_n else 2) as xtp,
            tc.tile_pool(name="h", bufs=1) as hp,
            tc.tile_pool(name="outsb", bufs=OUT_BUFS) as outp,
            tc.tile_pool(name="psA", bufs=PSA_BUFS, space="PSUM") as psA,
            tc.tile_pool(name="psB", bufs=PSB_BUFS, space="PSUM") as psB,
        ):
            # SBUF weight layouts mirror the DRAM group blocks:
            #   w1_sb[p, g*4096 + hk*512 + c] = w1[hk*128+p, g*512+c]
            #   w2_sb[p, (g*4+k)*H + c]       = w2[(g*4+k)*128+p, c]
            w1_sb = constp.tile([128, WG * H_TILES * 512], f16)
            w2_sb = constp.tile([128, F_TILES * H], f16)
            b1_sb = constp.tile([128, F_TILES], f32)
            b2_sb = constp.tile([1, H], f16)
            ones_sb = constp.tile([1, 512], f16)
            wvec_sb = constp.tile([128, nwv * 128], f32)

            def w1_slice(hk, fm):
                g, m = fm // 4, fm % 4
                off = g * (H_TILES * 512) + hk * 512 + m * 128
                return w1_sb[:, off : off + 128]

            def load_w1_group(g):
                nc.sync.dma_start(
                    w1_sb[:, g * (H_TILES * 512) : (g + 1) * (H_TILES * 512)],
                    w1_d[g],
                )

            def load_w2_group(g):
                sz = (F_TILES // WG) * H
                nc.sync.dma_start(w2_sb[:, g * sz : (g + 1) * sz], w2_d[g])

            def load_small(split=False):
                nc.sync.dma_start(b1_sb[:], b1_d[:])
                nc.vector.memset(ones_sb[:], 1.0)
                if with_b2:
                    nc.sync.dma_start(b2_sb[:], b2_d[:])
                if not split:
                    load_wvec()

            def load_wvec():
                nc.sync.dma_start(wvec_sb[:], wv_d[:])

            def emit_chunk(xt_sb, off, tc_sz, last=False, do_a=True, do_b=True,
                           split_head=False):
                # phase A: h^T = gelu(w1.T @ xT + b1), fp16.
                # split_head: the first 4 psum groups issue their hk 0-3
                # matmuls first, so the PE starts on the first half-MB of
                # xT/w1 while the second half is still in flight.
                h_sb = hp.tile([128, F_TILES * tc_sz], f16, name="h_sb")

                def mm_a(ps, fm, hk):
                    nc.tensor.matmul(
                        ps[:, :tc_sz],
                        w1_slice(hk, fm),
                        xt_sb[:, hk * tc_sz : (hk + 1) * tc_sz],
                        start=(hk == 0),
                        stop=(hk == H_TILES - 1),
                    )

                def act_a(ps, fm):
                    nc.scalar.activation(
                        h_sb[:, fm * tc_sz : (fm + 1) * tc_sz],
                        ps[:, :tc_sz],
                        mybir.ActivationFunctionType.Gelu_apprx_tanh,
                        bias=b1_sb[:, fm : fm + 1],
                    )

                head = min(PSA_BUFS, 4) if (do_a and split_head) else 0
                head_tiles = [psA.tile([128, 512], f32, name="psa")
                              for _ in range(head)]
                for h0, h1 in ((0, 2), (2, 4), (4, H_TILES)):
                    for fm in range(head):
                        for hk in range(h0, h1):
                            mm_a(head_tiles[fm], fm, hk)
                for fm in range(head):
                    act_a(head_tiles[fm], fm)
                for fm in range(head, F_TILES if do_a else 0):
                    ps = psA.tile([128, 512], f32, name="psa")
                    for hk in range(H_TILES):
                        mm_a(ps, fm, hk)
                    act_a(ps, fm)

                # phase B: y^T[hh, t] = w2[f, hh].T @ h^T[f, t] (+ b2.T @ 1),
                # tokens are the moving operand so the matmul cost scales
                # with the routed token count; combine on DVE, store y^T.
                for hm in range(H_TILES if do_b else 0):
                    ps = psB.tile([128, 512], f32, name="psb")
                    for fk in range(F_TILES):
                        nc.tensor.matmul(
                            ps[:, :tc_sz],
                            w2_sb[:, fk * H + hm * 128 : fk * H + (hm + 1) * 128],
                            h_sb[:, fk * tc_sz : (fk + 1) * tc_sz],
                            start=(fk == 0),
                            stop=(not with_b2 and fk == F_TILES - 1),
                        )
                    if with_b2:
                        nc.tensor.matmul(
                            ps[:, :tc_sz],
                            b2_sb[:, hm * 128 : (hm + 1) * 128],
                            ones_sb[:, :tc_sz],
                            start=False,
                            stop=True,
                        )
                    o_sb = outp.tile([128, 512], f32, name="o_sb")
                    if last and hm == H_TILES - 1:
                        # split the very last combine+store so the first half
                        # drains while the second is still being scaled
                        hh = tc_sz // 2
                        for lo, hi in ((0, hh), (hh, tc_sz)):
                            nc.vector.tensor_mul(
                                o_sb[:, lo:hi],
                                ps[:, lo:hi],
                                wvec_sb[:, off + lo : off + hi],
                            )
                            nc.sync.dma_start(
                                out_d[hm * 128 : (hm + 1) * 128,
                                      off + lo : off + hi],
                                o_sb[:, lo:hi],
                            )
                    else:
                        nc.vector.tensor_mul(
                            o_sb[:, :tc_sz],
                            ps[:, :tc_sz],
                            wvec_sb[:, off : off + tc_sz],
                        )
                        nc.sync.dma_start(
                            out_d[hm * 128 : (hm + 1) * 128, off : off + tc_sz],
                            o_sb[:, :tc_sz],
                        )

            if loop_n:
                # benchmark mode: weights resident, xT streamed per iteration
                for g in range(WG):
                    load_w1_group(g)
                for g in range(WG):
                    load_w2_group(g)
                load_small()
                import os as _os
                kw = {}
                if _os.environ.get("BENCH_STAGGER"):
                    kw["staggered_reset"] = True
                if _os.environ.get("BENCH_HINT"):
                    kw["hint_engines"] = (mybir.EngineType.PE,)
                do_a = not _os.environ.get("BENCH_BONLY")
                do_b = not _os.environ.get("BENCH_AONLY")
                pure = _os.environ.get("BENCH_PURE")  # "1": fixed stationary,
                # "2": rotating stationary. Pure back-to-back MM rate probe.
                with tc.For_i(0, loop_n, 1, **kw):
                    if pure:
                        for i in range(1024):
                            ws = (i % 8) * 128 if pure == "2" else 0
                            ps = psA.tile([128, 512], f32, name="psa")
                            nc.tensor.matmul(
                                ps[:],
                                w1_sb[:, ws : ws + 128],
                                w1_sb[:, 1024:1536],
                                start=True,
                                stop=True,
                            )
                        continue_body = None  # noqa: F841
                    else:
                        off = 0
                        for ci, tc_sz in enumerate(chunks):
                            xt_sb = xtp.tile(
                                [128, H_TILES * tc_sz], f16, name="xt_sb"
                            )
                            nc.sync.dma_start(
                                xt_sb[:],
                                xT_d[:, H_TILES * off : H_TILES * (off + tc_sz)],
                            )
                            emit_chunk(
                                xt_sb, off, tc_sz, last=(ci == n_chunks - 1),
                                do_a=do_a, do_b=do_b,
                            )
                            off += tc_sz
            else:
                # single-shot: DMA issue order IS the critical path.
                # Warmup matmuls on zeroed tiles keep the PE busy (and its
                # p-state ramping) while the first loads land.
                warm_src = constp.tile([128, 512 + 128], f16)
                nc.vector.memset(warm_src[:], 0.0)
                warm_ps = psA.tile([128, 512], f32, name="psa")
                xt_tiles = []
                offs = []
                off = 0
                for ci, tc_sz in enumerate(chunks):
                    xt_tiles.append(
                        xtp.tile([128, H_TILES * tc_sz], f16, name=f"xt{ci}")
                    )
                    offs.append(off)
                    off += tc_sz

                def load_xt(ci):
                    o, tc_sz = offs[ci], chunks[ci]
                    nc.sync.dma_start(
                        xt_tiles[ci][:],
                        xT_d[:, H_TILES * o : H_TILES * (o + tc_sz)],
                    )

                # first chunk's xT and w1 group 0 arrive as hk-pieces
                # (2+2+4) issued on BOTH HWDGE queues in parallel (xT on SP,
                # w1 on the still-idle Activation queue) — the SP sequencer's
                # ~650 ns per-DMA issue time is the startup serializer, so
                # splitting across queues is what actually moves the first
                # real matmul earlier (see split_head)
                for h0, h1 in ((0, 2), (2, 4), (4, H_TILES)):
                    nc.sync.dma_start(
                        xt_tiles[0][:, h0 * chunks[0] : h1 * chunks[0]],
                        xT_d[:, h0 * chunks[0] : h1 * chunks[0]],
                    )
                    nc.scalar.dma_start(
                        w1_sb[:, h0 * 512 : h1 * 512],
                        w1_d[0][:, h0 * 512 : h1 * 512],
                    )
                load_small(split=True)
                for w in range(WARMUP_MMS):
                    nc.tensor.matmul(
                        warm_ps[:],
                        warm_src[:, 512:640],
                        warm_src[:, 0:512],
                        start=True,
                        stop=True,
                    )
                for g in range(1, WG):
                    load_w1_group(g)
                for ci in range(1, n_chunks):
                    load_xt(ci)
                for g in range(WG):
                    load_w2_group(g)
                # wvec (0.5 MiB) is only needed at the first combine ~65 us
                # in; keeping it out of the early window frees DMA time for
                # the w1 groups the PE is actually waiting on
                load_wvec()
                for ci, tc_sz in enumerate(chunks):
                    emit_chunk(
                        xt_tiles[ci], offs[ci], tc_sz,
                        last=(ci == n_chunks - 1), split_head=(ci == 0),
                    )

    nc.compile()
    return nc


_NC_CACHE = {}


def _get_nc(chunks=CHUNKS_SPARSE, with_b2=True):
    key = (chunks, with_b2)
    if key not in _NC_CACHE:
        _NC_CACHE[key] = _build_nc(chunks, with_b2=with_b2)
    return _NC_CACHE[key]


def _route(probs, experts):
    """Per-core routed token indices and combine weights.

    Returns (idx_list, w_list): token indices (unique, sorted) routed to
    each expert and the summed probs for those tokens.
    """
    pf = np.asarray(probs, dtype=np.float32).reshape(K, T)
    ef = np.asarray(experts).reshape(K, T)
    idx_list, w_list = [], []
    for c in range(N_CORES):
        m = ef == c  # [K, T]
        sel = m.any(axis=0)
        idx = np.nonzero(sel)[0]
        w = (pf * m).sum(axis=0)[idx]
        idx_list.append(idx)
        w_list.append(w.astype(np.float32))
    return idx_list, w_list


def _prep_in_maps(x, probs, experts, w1, b1, w2, b2, chunks=CHUNKS_SPARSE,
                  route=None):
    cap = sum(chunks)
    nwv = (cap + 127) // 128
    x = np.asarray(x, dtype=np.float32).reshape(T, H)
    xT = np.ascontiguousarray(x.T).astype(np.float16)          # [H, T]
    w1f = np.asarray(w1, dtype=np.float32).astype(np.float16)  # [E, H, F]
    w2f = np.asarray(w2, dtype=np.float32).astype(np.float16)  # [E, F, H]
    b1f = np.asarray(b1, dtype=np.float32)                     # [E, F]
    b2f = np.asarray(b2, dtype=np.float32).astype(np.float16)  # [E, H]
    if route is None:
        route = _route(probs, experts)
    idx_list, w_list = route

    in_maps = []
    for c in range(N_CORES):
        idx, w = idx_list[c], w_list[c]
        n = len(idx)
        xsel = np.zeros((H, cap), dtype=np.float16)
        xsel[:, :n] = xT[:, idx]
        # per chunk [128p, hk, tc] <- xsel[hk*128+p, off+c], concat on free dim
        blocks = []
        off = 0
        for tc_sz in chunks:
            blk = (
                xsel[:, off : off + tc_sz]
                .reshape(H_TILES, 128, tc_sz)
                .transpose(1, 0, 2)
                .reshape(128, H_TILES * tc_sz)
            )
            blocks.append(blk)
            off += tc_sz
        xdr = np.ascontiguousarray(np.concatenate(blocks, axis=1))
        # [g, 128p, hk, 512c] <- w1[hk*128+p, g*512+c]
        w1dr = np.ascontiguousarray(
            w1f[c].reshape(H_TILES, 128, WG, 512)
            .transpose(2, 1, 0, 3)
            .reshape(WG, 128, H_TILES * 512)
        )
        # [g, 128p, k, 1024c] <- w2[(g*4+k)*128+p, c]
        kpg = F_TILES // WG
        w2dr = np.ascontiguousarray(
            w2f[c].reshape(WG, kpg, 128, H)
            .transpose(0, 2, 1, 3)
            .reshape(WG, 128, kpg * H)
        )
        wv = np.zeros(nwv * 128, dtype=np.float32)
        wv[:n] = w
        in_maps.append(
            {
                "xT": xdr,
                "w1": w1dr,
                "w2": w2dr,
                "b1T": np.ascontiguousarray(b1f[c].reshape(F // 128, 128).T),
                "b2r": b2f[c].reshape(1, H),
                # replicated across partitions for the DVE combine
                "wvec": np.ascontiguousarray(
                    np.broadcast_to(wv, (128, nwv * 128))
                ),
            }
        )
    return in_maps


def _unshard(results, route):
    idx_list, _ = route
    out = np.zeros((T, H), dtype=np.float32)
    for c in range(N_CORES):
        idx = idx_list[c]
        # device output is y^T [H, cap]
        out[idx] += results[c]["out"][:, : len(idx)].T
    return out.reshape(B, S, H)


def _pick_chunks(max_n):
    """Smallest 32-granular capacity >= max routed count, as <=512 chunks.

    The reference distribution peaks at ~992 routed tokens per expert (mean
    960, sigma 27) -> (512, 480). Anything else still compiles a correct
    variant (compile cached per chunk tuple)."""
    capq = max(128, ((max_n + 31) // 32) * 32)
    full, rem = divmod(capq, 512)
    return (512,) * full + ((rem,) if rem else ())


def kernel(x, probs, experts, w1, b1, w2, b2):
    route = _route(probs, experts)
    max_n = max(len(i) for i in route[0])
    chunks = _pick_chunks(max_n)
    nc = _get_nc(chunks, with_b2=bool(np.any(np.asarray(b2))))
    in_maps = _prep_in_maps(
        x, probs, experts, w1, b1, w2, b2, chunks=chunks, route=route
    )
    res = run_bass_kernel_spmd(nc, in_maps, core_ids=list(range(N_CORES)))
    return _unshard(res.results, route)
